# revision 75
# baseline (speedup 1.0000x reference)
"""GQA causal attention (RoPE) for TRN2, 8-core data+tensor parallel.

Sharding: core c in [0,8) handles batch b = c//4 and kv-head group g = c%4
(kv heads {2g, 2g+1}, q heads {4g..4g+3}).  wq/wk/wv column-sharded,
wo row-sharded by head group; host sums the 4 partial wo outputs per batch.

Device layouts (feature-major, "T" = transposed vs reference):
  xt   [DIM, S]      activations, d on partitions
  QT   [128, S]      per q head (head_dim on partitions)
  KT   [128, S]      per kv head
  V    [128k, 256]   natural (position on partitions), 16 k-tiles
  scoresT[k, q]      so softmax denominator is a partition-dim sum (ones matmul)
  attnT [128d, S]    per head -> wo matmul -> outT [DIM, S] (host transposes)

RoPE on [d, s] tiles: out = qt * C + swap_pairs(qt) * S~, with the pair swap
done by a permutation matmul on the PE and C/S~ tables prebuilt on host.

Optimizations vs the 325us baseline (now ~309us):
  - All inputs arrive as FEW BIG DMAs from host-pre-tiled DRAM layouts
    (>=4KB per-partition rows).  Per-queue DMA throughput is dispatch-
    limited (~565-667ns per dma_start) and packet-limited, so the old
    ~100-dispatch startup starved the PE for ~16us.
  - Startup: chunk 0's K/Q/V projections are interleaved per d-group so
    the PE's fresh-byte consumption (~200B/ns) stays under the DMA
    supply; its weight/xt pieces stream on BOTH HWDGE queues in exact
    consumption order (one queue sustains only ~250-280B/ns).  Later
    chunks' xt, wo, cos/sin dispatch from Act-queue program points
    anchored behind eviction copies, so they cannot steal early HBM
    bandwidth.
  - Phase-B softmax normalize: den (PE, in-stream) -> ACT Ln/Exp recip
    -> gpsimd DRAM-bounce broadcast -> DVE mul, i.e. no PE instruction
    in the chain (the old PE broadcast matmul stalled the PE ~1.2us at
    every head boundary waiting on the ACT queue).  Phase-A norms and
    the last (latency-exposed) norm keep the PE-broadcast form: the PE
    has slack there, and the gpsimd SWDGE round trip is ~6-8us.
  - wo evictions alternate ACT/DVE (a single engine queue serialized
    wop bank recycling); mid-phase outT DMAs ride the idle SP queue,
    the tail burst cycles sync/scalar/gpsimd.
  - Tail: the last chunk's 16 wo tiles run in 2 waves of 8 PSUM banks
    borrowed from the quiescent attn pools, pre-accumulating heads 0-2
    while head 3's normalize completes.

Scheduling (PE is in-order; emission order = execution order):
  - The projection phase is PE-bound while ACT/DVE idle, and the attention
    phase is bound by ACT (exp) / DVE (dac, evictions).  So attention for
    chunks 0 and 1 (and all RoPE) is broken into micro-tasks that are pumped
    between projection PSUM groups of chunks 2/3.
  - Attention chunks 2/3 run after, with scores emitted DEPTH ahead of
    their attnV matmuls, and wo tiles of ready chunks (0,1 then 3) as PE
    filler between heads to cover the exp chains.
  - Causal mask is applied by accumulating a -1e30 strict-upper block into
    the scores PSUM on the PE itself (no extra engine in the chain).
"""

import json
from collections import deque

import numpy as np
import ml_dtypes

import concourse.bass as bass
import concourse.mybir as mybir
import concourse.tile as tile
import concourse.bass2jax as bass2jax
import concourse.bass_utils as bass_utils
from concourse.bass_utils import run_bass_kernel_spmd


def _split_waits(bir_json: bytes) -> bytes:
    """This walrus build accepts at most ONE sync-wait per instruction (any
    opcode). Tile emits up to ~11. Hoist excess waits onto single-wait Drain
    fillers inserted just before the instruction on the same engine —
    same-engine program order makes this semantically identical."""
    j = json.loads(bir_json)
    changed = False
    for fn in j["functions"]:
        for b in fn["blocks"]:
            out = []
            for ins in b["instructions"]:
                si = ins.get("sync_info")
                ow = si.get("on_wait") if si else None
                if ow and len(ow) > 1:
                    changed = True
                    for k, w in enumerate(ow[:-1]):
                        out.append({
                            "debug": ins.get("debug", 0),
                            "engine": ins["engine"],
                            "ins": [], "outs": [],
                            "name": f"{ins['name']}-w{k}",
                            "opcode": "Drain",
                            "is_reset_sema": False,
                            "sync_info": {"on_update": [], "on_wait": [w]},
                        })
                    si["on_wait"] = [ow[-1]]
                out.append(ins)
            b["instructions"] = out
    return json.dumps(j).encode() if changed else bir_json


_ORIG_COMPILE = bass_utils.compile_bir_kernel


def _patched_compile(bir_json, tmpdir, neff_name="file.neff"):
    return _ORIG_COMPILE(_split_waits(bir_json), tmpdir, neff_name=neff_name)


if getattr(bass2jax.compile_bir_kernel, "__name__", "") != "_patched_compile":
    bass2jax.compile_bir_kernel = _patched_compile
    bass_utils.compile_bir_kernel = _patched_compile

BF16 = mybir.dt.bfloat16
F32 = mybir.dt.float32
Exp = mybir.ActivationFunctionType.Exp
Ln = mybir.ActivationFunctionType.Ln

B, S, DIM = 2, 2048, 2048
N_HEADS, N_KV_HEADS = 16, 8
HEAD_DIM, HALF = 128, 64
N_CORES = 8
QH, KVH = 4, 2            # q / kv heads per core
QW, KW = QH * HEAD_DIM, KVH * HEAD_DIM   # 512, 256
SCALE = 1.0 / float(np.sqrt(HEAD_DIM))

DT = DIM // 128           # 16 contraction tiles for projections
NSC = S // 512            # 4 s-chunks
NKT = S // 128            # 16 k tiles
NET = DIM // 128          # 16 output-feature tiles
DEPTH = 3                 # phase-B: score matmuls emitted ahead of attnV

_BUILT = {}


def _build(nc):
    # Pre-tiled DRAM layouts (see prepare_in_maps): every load is 1-2 big
    # DMAs with >=4KB contiguous per-partition rows.
    xtp = nc.dram_tensor("xtp", [128, NSC * DT * 512], BF16, kind="ExternalInput").ap()
    wqp = nc.dram_tensor("wqp", [128, DT * QW], BF16, kind="ExternalInput").ap()
    wkp = nc.dram_tensor("wkp", [128, DT * KW], BF16, kind="ExternalInput").ap()
    wvp = nc.dram_tensor("wvp", [128, DT * KW], BF16, kind="ExternalInput").ap()
    wop = nc.dram_tensor("wop", [128, QH * DIM], BF16, kind="ExternalInput").ap()
    cosb = nc.dram_tensor("cosb", [HEAD_DIM, S], BF16, kind="ExternalInput").ap()
    sinb = nc.dram_tensor("sinb", [HEAD_DIM, S], F32, kind="ExternalInput").ap()
    # pswp | ident | maskn | ones | onerow(row 0) merged into one table load
    tbl = nc.dram_tensor("tbl", [HEAD_DIM, 513], BF16, kind="ExternalInput").ap()
    outT = nc.dram_tensor("outT", [DIM, S], BF16, kind="ExternalOutput").ap()
    # DRAM bounce buffers for partition-broadcast of per-position reciprocals
    rscr = [nc.dram_tensor(f"rscr{i}", [1, 512], F32).ap() for i in range(NSC * QH)]

    with tile.TileContext(nc) as tc:
        with (
            tc.tile_pool(name="persist", bufs=1) as pp,
            tc.tile_pool(name="trans", bufs=2) as tp,
        ):
            # ---- startup DMA plan.  Act queue: wk halves, wv, wq, wo.
            # SP queue: xt chunk halves in chunk order (c2/c3 tile-recycle
            # gated, which is fine — they're needed much later).  Pool
            # (gpsimd SWDGE): tables + cos/sin, later the norm broadcasts.
            # Dependency tracking is tile-granular (a consumer waits on ALL
            # writers of its tile), so anything consumed piecewise gets one
            # SBUF tile per DMA: wk/wq split at d=8, xt chunks in d-quarters.
            # The SP HWDGE ring starts ~2us before Act's, so the two tensors
            # gating the first matmuls (wk half 0, xt0 quarter 0) lead the SP
            # queue; bulk weights ride Act.
            # Chunk 0's projections are interleaved K/Q/V per d-group (see
            # below) so its weights and xt arrive as d-quarter sets on the SP
            # queue in exactly consumption order: each 1.5MB set feeds ~4.3us
            # of PE, matching the ~300B/ns DMA supply.  Later chunks' xt and
            # wo ride the Act queue (its HWDGE ring starts ~2us later).
            wk_sb = [pp.tile([128, 4 * KW], BF16, tag=f"wk{g}", name=f"wk_sb{g}")
                     for g in range(4)]
            wq_sb = [pp.tile([128, 4 * QW], BF16, tag=f"wq{g}", name=f"wq_sb{g}")
                     for g in range(4)]
            wv_sb = [pp.tile([128, 4 * KW], BF16, tag=f"wv{g}", name=f"wv_sb{g}")
                     for g in range(4)]
            # Alternate the critical stream across BOTH HWDGE queues in
            # consumption order: one queue sustains only ~250-280B/ns while
            # the interleaved chunk-0 projections consume ~350B/ns.
            xa = [[None] * 4 for _ in range(NSC)]
            for g in range(4):
                nc.sync.dma_start(wk_sb[g][:], wkp[:, g * 1024:(g + 1) * 1024])
                t = tp.tile([128, 2048], BF16, tag=f"xtq{g}", bufs=2, name=f"xa0_{g}")
                nc.sync.dma_start(t[:], xtp[:, g * 2048:(g + 1) * 2048])
                xa[0][g] = t
                nc.scalar.dma_start(wq_sb[g][:], wqp[:, g * 2048:(g + 1) * 2048])
                nc.scalar.dma_start(wv_sb[g][:], wvp[:, g * 1024:(g + 1) * 1024])

            # xt chunks 1-3, wo, cos, sin are dispatched from the Act queue at
            # anchored program points inside phase A (the Act engine only
            # reaches those dma_starts after earlier copies execute), so their
            # transfers cannot steal HBM bandwidth from the critical chunk-0
            # set streaming on SP.  Tiles are allocated at the anchor points
            # to keep pool-rotation order correct.
            wo_sb = pp.tile([128, QH * DIM], BF16, tag="wo", name="wo_sb")

            def load_chunk(sc):
                for q in range(4):
                    t = tp.tile([128, 2048], BF16, tag=f"xtq{q}", bufs=2,
                                name=f"xa{sc}_{q}")
                    nc.scalar.dma_start(
                        t[:], xtp[:, sc * 8192 + q * 2048: sc * 8192 + (q + 1) * 2048])
                    xa[sc][q] = t

            tbl_sb = pp.tile([HEAD_DIM, 513], BF16, tag="tbl", name="tbl_sb")
            nc.gpsimd.dma_start(tbl_sb[:], tbl[:])
            cos_sb = pp.tile([HEAD_DIM, S], BF16, tag="cos", name="cos_sb")
            sin_sb = pp.tile([HEAD_DIM, S], F32, tag="sin", name="sin_sb")

            pswp_sb = tbl_sb[:, 0:128]
            ident_sb = tbl_sb[:, 128:256]
            maskn_sb = tbl_sb[:, 256:384]
            ones_sb = tbl_sb[:, 384:385]
            onerow_sb = tbl_sb[0:1, 385:513]

            def xs(sc, d):
                t = xa[sc][d // 4]
                return t[:, (d % 4) * 512:(d % 4 + 1) * 512]

            def wqs(d, h):
                return wq_sb[d // 4][:, (d % 4) * QW + h * 128:
                                     (d % 4) * QW + (h + 1) * 128]

            def wks(d, kv):
                return wk_sb[d // 4][:, (d % 4) * KW + kv * 128:
                                     (d % 4) * KW + (kv + 1) * 128]

            def wvs(d):
                return wv_sb[d // 4][:, (d % 4) * KW:(d % 4 + 1) * KW]

            def wos(h, et):
                return wo_sb[:, h * DIM + et * 128: h * DIM + (et + 1) * 128]

            # persistent intermediates
            qtu = [pp.tile([128, S], BF16, tag=f"qtu{h}", name=f"qtu{h}") for h in range(QH)]
            ktu = [pp.tile([128, S], BF16, tag=f"ktu{k}", name=f"ktu{k}") for k in range(KVH)]
            qtr = [pp.tile([128, S], BF16, tag=f"qtr{h}", name=f"qtr{h}") for h in range(QH)]
            ktr = [pp.tile([128, S], BF16, tag=f"ktr{k}", name=f"ktr{k}") for k in range(KVH)]
            v_sb = [pp.tile([128, KW], BF16, tag=f"v{st}", name=f"v{st}") for st in range(NKT)]
            attnT = [pp.tile([128, S], BF16, tag=f"attnT{h}", name=f"attnT{h}") for h in range(QH)]

            def norm_v3(qc, h, den_src, attn_ps):
                """normalize with no PE instruction in the chain: ACT
                1/x = exp(-ln(x)) -> gpsimd DRAM-bounce broadcast DMA ->
                DVE mul.  (DVE InstReciprocal on a [1,512] tile is ~2.6us
                of single-lane work and clogs the DVE queue; SBUF->SBUF
                partition-broadcast is illegal, so bounce through DRAM.)"""
                qsl = slice(qc * 512, (qc + 1) * 512)
                lnd = tp.tile([1, 512], F32, tag="lnd", bufs=2, name=f"lnd{qc}_{h}")
                nc.scalar.activation(lnd[:], den_src, Ln)
                recip = tp.tile([1, 512], F32, tag="recip", bufs=2, name=f"recip{qc}_{h}")
                nc.scalar.activation(recip[:], lnd[:], Exp, scale=-1.0)
                scr = rscr[qc * QH + h]
                nc.gpsimd.dma_start(scr[:], recip[:])
                rb = tp.tile([128, 512], F32, tag="rb", bufs=2, name=f"rb{qc}_{h}")
                bc = bass.AP(tensor=scr.tensor, offset=scr.offset,
                             ap=[[0, 128]] + list(scr.ap[1:]))
                nc.gpsimd.dma_start(rb[:], bc)
                nc.vector.tensor_mul(attnT[h][:, qsl], attn_ps[:], rb[:])

            # ========== Phase A: projections + interleaved attn(0,1)+rope ==
            with tc.tile_pool(name="pmA", bufs=1, space="PSUM") as pm:
                # psum/partition budget (16KB): qk 2x2K, v 2x2K(1K used),
                # shp 1x2K, asc 2x2K (scores for interleaved attn; den rides
                # row 0 of an asc tile), aps 1x2K
                tasks = deque()

                def pump(n):
                    for _ in range(n):
                        if not tasks:
                            return
                        t = tasks.popleft()
                        if next(t, None) is not None:
                            tasks.appendleft(t)

                def rope_one(src, dst, ssl, nm):
                    # (a half-swap via SBUF->SBUF DMAs was tried — it saves
                    # ~2us of PE but lengthens the rope chains that phase B's
                    # first scores wait on; the PE permutation matmul wins)
                    shp = pm.tile([128, 512], F32, tag="shp", bufs=1, name=f"shp{nm}")
                    nc.tensor.matmul(shp[:], pswp_sb, src[:, ssl], start=True, stop=True)
                    t1 = tp.tile([128, 512], BF16, tag="t1", bufs=3, name=f"rt1{nm}")
                    nc.vector.tensor_mul(t1[:], src[:, ssl], cos_sb[:, ssl])
                    t2 = tp.tile([128, 512], BF16, tag="t2", bufs=3, name=f"rt2{nm}")
                    nc.vector.tensor_mul(t2[:], shp[:], sin_sb[:, ssl])
                    nc.vector.tensor_add(dst[:, ssl], t1[:], t2[:])

                def rope_gen(sc):
                    ssl = slice(sc * 512, (sc + 1) * 512)
                    rope_one(qtu[0], qtr[0], ssl, f"q0_{sc}")
                    yield 1
                    for kv in range(KVH):
                        rope_one(ktu[kv], ktr[kv], ssl, f"k{kv}_{sc}")
                        yield 1
                    for h in range(1, QH):
                        rope_one(qtu[h], qtr[h], ssl, f"q{h}_{sc}")
                        yield 1

                def attn_gen(qc, h):
                    """micro-task generator: one kt step (score+exp+dac and
                    the previous step's attnV) per yield."""
                    qsl = slice(qc * 512, (qc + 1) * 512)
                    nkt = 4 * qc + 4
                    kv = h // 2
                    attn_ps = pm.tile([128, 512], F32, tag="aps", bufs=1, name=f"Aattn{qc}_{h}")
                    dac = tp.tile([128, 512], BF16, tag="dac", bufs=2, name=f"Adac{qc}_{h}")
                    pend = []

                    def attn_v(kt, off, span, pt):
                        nc.tensor.matmul(attn_ps[:, off:],
                                         v_sb[kt][:, kv * 128:(kv + 1) * 128],
                                         pt[:, :span], start=(kt == 0),
                                         stop=(kt == nkt - 1))

                    for kt in range(nkt):
                        off = max(0, 128 * kt - 512 * qc)
                        span = 512 - off
                        diag = kt >= 4 * qc
                        scps = pm.tile([128, 512], F32, tag="asc", bufs=2, name=f"Asc{qc}_{h}_{kt}")
                        nc.tensor.matmul(scps[:, :span], ktr[kv][:, kt * 128:(kt + 1) * 128],
                                         qtr[h][:, qc * 512 + off:(qc + 1) * 512],
                                         start=True, stop=not diag)
                        if diag:
                            nc.tensor.matmul(scps[:, :128], ident_sb, maskn_sb,
                                             start=False, stop=True)
                        pt = tp.tile([128, 512], BF16, tag="pt", bufs=6, name=f"Apt{qc}_{h}_{kt}")
                        nc.scalar.activation(pt[:, :span], scps[:, :span], Exp, scale=SCALE)
                        if kt == 0:
                            nc.vector.tensor_copy(dac[:], pt[:])
                        else:
                            nc.vector.tensor_add(dac[:, off:], dac[:, off:], pt[:, :span])
                        pend.append((kt, off, span, pt))
                        if len(pend) > 1:
                            attn_v(*pend.pop(0))
                        yield 1
                    while pend:
                        attn_v(*pend.pop(0))
                    # den rides in row 0 of an asc-tag psum tile
                    dent = pm.tile([128, 512], F32, tag="asc", bufs=2, name=f"Aden{qc}_{h}")
                    nc.tensor.matmul(dent[0:1, :], ones_sb, dac[:], start=True, stop=True)
                    yield 1
                    # phase A normalizes via PE outer-product broadcast: the
                    # PE has slack here (projections dominate), and the
                    # gpsimd DRAM bounce would serialize the end-of-phase
                    # drain where several norms flush back-to-back.
                    qsl2 = slice(qc * 512, (qc + 1) * 512)
                    lnd = tp.tile([1, 512], F32, tag="lnd", bufs=2, name=f"Alnd{qc}_{h}")
                    nc.scalar.activation(lnd[:], dent[0:1, :], Ln)
                    recipb = tp.tile([1, 512], BF16, tag="frecip", bufs=2,
                                     name=f"Afrecip{qc}_{h}")
                    nc.scalar.activation(recipb[:], lnd[:], Exp, scale=-1.0)
                    bc_ps = pm.tile([128, 512], F32, tag="asc", bufs=2, name=f"Abc{qc}_{h}")
                    nc.tensor.matmul(bc_ps[:], onerow_sb, recipb[:], start=True, stop=True)
                    rbs = tp.tile([128, 512], F32, tag="rb", bufs=2, name=f"Arbs{qc}_{h}")
                    nc.scalar.copy(rbs[:], bc_ps[:])
                    nc.vector.tensor_mul(attnT[h][:, qsl2], attn_ps[:], rbs[:])
                    yield 1

                def proj_q(sc, ssl, after=0):
                    for h in range(QH):
                        ps = pm.tile([128, 512], F32, tag="qk", bufs=2, name=f"qps{sc}_{h}")
                        for d in range(DT):
                            nc.tensor.matmul(ps[:], wqs(d, h), xs(sc, d),
                                             start=(d == 0), stop=(d == DT - 1))
                        nc.scalar.copy(qtu[h][:, ssl], ps[:])
                        pump(after)

                def proj_k(sc, ssl, after=0):
                    for kv in range(KVH):
                        ps = pm.tile([128, 512], F32, tag="qk", bufs=2, name=f"kps{sc}_{kv}")
                        for d in range(DT):
                            nc.tensor.matmul(ps[:], wks(d, kv), xs(sc, d),
                                             start=(d == 0), stop=(d == DT - 1))
                        nc.scalar.copy(ktu[kv][:, ssl], ps[:])
                        pump(after)

                def proj_v(sc, after=0):
                    for sv in range(4):
                        st = sc * 4 + sv
                        ps = pm.tile([128, KW], F32, tag="v", bufs=2, name=f"vps{st}")
                        for d in range(DT):
                            nc.tensor.matmul(ps[:], xs(sc, d)[:, sv * 128:(sv + 1) * 128],
                                             wvs(d), start=(d == 0), stop=(d == DT - 1))
                        nc.scalar.copy(v_sb[st][:], ps[:])
                        pump(after)

                # c0: interleaved K/Q(all 4 heads)/V(sv0,sv1) per d-group, so
                # the PE's consumption (~200B/ns of fresh bytes) stays below
                # the DMA supply — K alone would eat xt at ~300B/ns and
                # stall.  Accumulators borrow every free PSUM bank: K in qk,
                # Q0/Q1 in asc, Q2 in aps, Q3 in shp, V0/V1 in v (no rope or
                # attn tasks exist yet).  Pass 2 (V2,V3) re-reads SBUF xt.
                ssl0 = slice(0, 512)
                kps = [pm.tile([128, 512], F32, tag="qk", bufs=2, name=f"kps0_{kv}")
                       for kv in range(KVH)]
                qps = [pm.tile([128, 512], F32, tag="asc", bufs=2, name=f"qps0_{h}")
                       for h in range(2)]
                qps.append(pm.tile([128, 512], F32, tag="aps", bufs=1, name="qps0_2"))
                qps.append(pm.tile([128, 512], F32, tag="shp", bufs=1, name="qps0_3"))
                vps01 = [pm.tile([128, KW], F32, tag="v", bufs=2, name=f"vps0_{sv}")
                         for sv in range(2)]
                def c0_k(dg):
                    for kv in range(KVH):
                        for d in range(4 * dg, 4 * dg + 4):
                            nc.tensor.matmul(kps[kv][:], wks(d, kv), xs(0, d),
                                             start=(d == 0), stop=(d == DT - 1))

                def c0_qv(dg):
                    ds = range(4 * dg, 4 * dg + 4)
                    for h in range(QH):
                        for d in ds:
                            nc.tensor.matmul(qps[h][:], wqs(d, h), xs(0, d),
                                             start=(d == 0), stop=(d == DT - 1))
                    for sv in range(2):
                        for d in ds:
                            nc.tensor.matmul(vps01[sv][:], xs(0, d)[:, sv * 128:(sv + 1) * 128],
                                             wvs(d), start=(d == 0), stop=(d == DT - 1))

                # K for dg0+dg1 first (SP-supplied) — the Act ring starts
                # ~2.5us after SP's, so the first wq/wv arrive later; the
                # extra K work covers that window.
                c0_k(0)
                c0_k(1)
                c0_qv(0)
                c0_k(2)
                c0_qv(1)
                c0_k(3)
                c0_qv(2)
                c0_qv(3)
                for kv in range(KVH):
                    nc.scalar.copy(ktu[kv][:, ssl0], kps[kv][:])
                # anchor: Act has now executed its first copies, the SP
                # stream is nearly drained — release cos/sin + xt chunk 1.
                nc.scalar.dma_start(cos_sb[:], cosb[:])
                nc.scalar.dma_start(sin_sb[:], sinb[:])
                load_chunk(1)
                for h in range(QH):
                    nc.scalar.copy(qtu[h][:, ssl0], qps[h][:])
                for sv in range(2):
                    nc.scalar.copy(v_sb[sv][:], vps01[sv][:])
                # pass 2: V2,V3 from SBUF-resident xt
                vps23 = [pm.tile([128, KW], F32, tag="v", bufs=2, name=f"vps0_{sv}")
                         for sv in (2, 3)]
                for dg in range(4):
                    ds = range(4 * dg, 4 * dg + 4)
                    for i, sv in enumerate((2, 3)):
                        for d in ds:
                            nc.tensor.matmul(vps23[i][:], xs(0, d)[:, sv * 128:(sv + 1) * 128],
                                             wvs(d), start=(d == 0), stop=(d == DT - 1))
                for i, sv in enumerate((2, 3)):
                    nc.scalar.copy(v_sb[sv][:], vps23[i][:])
                # c1: queue rope(c0)
                tasks.append(rope_gen(0))
                load_chunk(2)
                wo_half = QH * DIM // 2
                nc.scalar.dma_start(wo_sb[:, 0:wo_half], wop[:, 0:wo_half])
                nc.scalar.dma_start(wo_sb[:, wo_half:], wop[:, wo_half:])
                ssl1 = slice(512, 1024)
                proj_q(1, ssl1, after=1)
                proj_k(1, ssl1, after=1)
                proj_v(1, after=1)
                # c2: queue rope(c1) then attn(0)
                tasks.append(rope_gen(1))
                load_chunk(3)
                for h in range(QH):
                    tasks.append(attn_gen(0, h))
                ssl2 = slice(1024, 1536)
                proj_q(2, ssl2, after=4)
                proj_k(2, ssl2, after=4)
                # rope(c2) queued AHEAD of attn(1): phase B now starts with
                # attn chunk 2, whose first scores need qtr/ktr chunk 2 —
                # pumping it early keeps it clear of attn(1)'s DVE backlog.
                tasks.append(rope_gen(2))
                for h in range(QH):
                    tasks.append(attn_gen(1, h))
                proj_v(2, after=4)
                # c3: rope(c3) units are emitted inline right after the
                # eviction each one depends on
                ssl3 = slice(1536, 2048)
                for h in range(QH):
                    ps = pm.tile([128, 512], F32, tag="qk", bufs=2, name=f"qps3_{h}")
                    for d in range(DT):
                        nc.tensor.matmul(ps[:], wqs(d, h), xs(3, d),
                                         start=(d == 0), stop=(d == DT - 1))
                    nc.scalar.copy(qtu[h][:, ssl3], ps[:])
                    rope_one(qtu[h], qtr[h], ssl3, f"q{h}_3")
                    pump(5)
                for kv in range(KVH):
                    ps = pm.tile([128, 512], F32, tag="qk", bufs=2, name=f"kps3_{kv}")
                    for d in range(DT):
                        nc.tensor.matmul(ps[:], wks(d, kv), xs(3, d),
                                         start=(d == 0), stop=(d == DT - 1))
                    nc.scalar.copy(ktu[kv][:, ssl3], ps[:])
                    rope_one(ktu[kv], ktr[kv], ssl3, f"k{kv}_3")
                    pump(5)
                proj_v(3, after=7)
                while tasks:
                    pump(1)

            # ========== Phase B: attn(3), attn(2) + wo tiles ==========
            wo_ctr = [0]

            def wo_evict(qc2, et, wo_ps, qcycle=False):
                # evictions alternate ACT/DVE so neither queue serializes the
                # wop bank recycling.  Mid-phase outT DMAs ride the otherwise-
                # idle SP queue (~1.1us/tile turnaround is plenty there); the
                # tail burst cycles sync/scalar/gpsimd so 16 back-to-back
                # writes don't back up the stage pool.
                qsl = slice(qc2 * 512, (qc2 + 1) * 512)
                stage = tp.tile([128, 512], BF16, tag="stage", bufs=8,
                                name=f"stage{qc2}_{et}")
                wo_ctr[0] += 1
                if wo_ctr[0] % 2:
                    nc.scalar.copy(stage[:], wo_ps[:])
                else:
                    nc.vector.tensor_copy(stage[:], wo_ps[:])
                eng = (nc.sync, nc.scalar, nc.gpsimd)[wo_ctr[0] % 3] if qcycle else nc.sync
                eng.dma_start(outT[et * 128:(et + 1) * 128, qsl], stage[:])

            with (
                tc.tile_pool(name="scp", bufs=4, space="PSUM") as scp,
                tc.tile_pool(name="attnp", bufs=2, space="PSUM") as attnp,
                tc.tile_pool(name="wop", bufs=2, space="PSUM") as wop,
            ):
                def wo_tiles(pairs, qcycle=False):
                    for qc2, et in pairs:
                        qsl = slice(qc2 * 512, (qc2 + 1) * 512)
                        wo_ps = wop.tile([128, 512], F32, tag="wo", name=f"wops{qc2}_{et}")
                        for h in range(QH):
                            nc.tensor.matmul(wo_ps[:], wos(h, et), attnT[h][:, qsl],
                                             start=(h == 0), stop=(h == QH - 1))
                        wo_evict(qc2, et, wo_ps, qcycle=qcycle)

                def attn_chunk(qc, fph, post_den=(), last=False):
                    # fph: per-head lists of (qc2, et) wo filler tiles;
                    # post_den: fillers emitted between the last head's den
                    # matmul and its normalize consumers.  Each head's
                    # normalize is deferred into the NEXT head's kt loop so
                    # the DVE mul never head-of-line-blocks the queue.
                    nkt = 4 * qc + 4
                    pending = [None]

                    def flush_norm():
                        if pending[0] is not None:
                            pending[0]()
                            pending[0] = None

                    for h in range(QH):
                        kv = h // 2
                        attn_ps = attnp.tile([128, 512], F32, tag="attn", name=f"attn{qc}_{h}")
                        dac = tp.tile([128, 512], BF16, tag="dac", bufs=2, name=f"dac{qc}_{h}")

                        def attn_v(kt, off, span, pt):
                            nc.tensor.matmul(attn_ps[:, off:],
                                             v_sb[kt][:, kv * 128:(kv + 1) * 128],
                                             pt[:, :span], start=(kt == 0),
                                             stop=(kt == nkt - 1))

                        # spread wo fillers through the kt loop, including two
                        # right at head start (kt 1 and 3): they cover the PE
                        # bubble while the first exps and the previous head's
                        # bank recycling catch up.
                        fillq = deque(fph[h])
                        stride = max(2, (nkt - 4) // max(1, max(1, len(fillq) - 2)))
                        pend = []
                        for kt in range(nkt):
                            off = max(0, 128 * kt - 512 * qc)
                            span = 512 - off
                            diag = kt >= 4 * qc
                            scps = scp.tile([128, 512], F32, tag="sc", name=f"sc{qc}_{h}_{kt}")
                            nc.tensor.matmul(scps[:, :span], ktr[kv][:, kt * 128:(kt + 1) * 128],
                                             qtr[h][:, qc * 512 + off:(qc + 1) * 512],
                                             start=True, stop=not diag)
                            if diag:
                                nc.tensor.matmul(scps[:, :128], ident_sb, maskn_sb,
                                                 start=False, stop=True)
                            pt = tp.tile([128, 512], BF16, tag="pt", bufs=6, name=f"pt{qc}_{h}_{kt}")
                            nc.scalar.activation(pt[:, :span], scps[:, :span], Exp, scale=SCALE)
                            if kt == 0:
                                nc.vector.tensor_copy(dac[:], pt[:])
                            else:
                                nc.vector.tensor_add(dac[:, off:], dac[:, off:], pt[:, :span])
                            if kt == 2:
                                flush_norm()
                            pend.append((kt, off, span, pt))
                            if len(pend) > DEPTH:
                                attn_v(*pend.pop(0))
                            if fillq and (kt == 1 or kt == 3 or
                                          (kt >= 4 and (kt - 4) % stride == 0)):
                                wo_tiles([fillq.popleft()])
                        while pend:
                            attn_v(*pend.pop(0))
                        flush_norm()

                        wo_tiles(list(fillq))

                        dent = scp.tile([128, 512], F32, tag="sc", name=f"den{qc}_{h}")
                        nc.tensor.matmul(dent[0:1, :], ones_sb, dac[:], start=True, stop=True)
                        if h == QH - 1:
                            wo_tiles(post_den)
                            if last:
                                # the very last norm is latency-exposed (the
                                # tail's h3 matmuls wait on it): PE-broadcast
                                # (~3us chain) instead of the gpsimd bounce
                                # (~8us: SWDGE gen + sem props per hop).
                                qsl3 = slice(qc * 512, (qc + 1) * 512)
                                lnd = tp.tile([1, 512], F32, tag="lnd", bufs=2,
                                              name=f"flnd{qc}_{h}")
                                nc.scalar.activation(lnd[:], dent[0:1, :], Ln)
                                recipb = tp.tile([1, 512], BF16, tag="frecip", bufs=2,
                                                 name=f"ffrecip{qc}_{h}")
                                nc.scalar.activation(recipb[:], lnd[:], Exp, scale=-1.0)
                                bc_ps = wop.tile([128, 512], F32, tag="wo",
                                                 name=f"fbc{qc}_{h}")
                                nc.tensor.matmul(bc_ps[:], onerow_sb, recipb[:],
                                                 start=True, stop=True)
                                rbs = tp.tile([128, 512], F32, tag="rb", bufs=2,
                                              name=f"frbs{qc}_{h}")
                                nc.scalar.copy(rbs[:], bc_ps[:])
                                nc.vector.tensor_mul(attnT[h][:, qsl3], attn_ps[:], rbs[:])
                            else:
                                norm_v3(qc, h, dent[0:1, :], attn_ps)
                        else:
                            def mk_norm(h=h, dent=dent, attn_ps=attn_ps):
                                norm_v3(qc, h, dent[0:1, :], attn_ps)
                            pending[0] = mk_norm

                # chunk 2 first: its scores need no chunk-3 rope at all, so
                # phase B starts without waiting on phase A's DVE drain;
                # chunk 2's wo tiles then become fillers inside chunk 3, and
                # the tail is chunk 3's wo.
                A32 = [(0, et) for et in range(NET)] + [(1, et) for et in range(NET)]
                C16 = [(2, et) for et in range(NET)]
                attn_chunk(2, [[], A32[0:9], A32[9:18], A32[18:26]])
                attn_chunk(3, [A32[26:32], C16[0:5], C16[5:10], C16[10:13]],
                           post_den=C16[13:16], last=True)

                # Tail: the 16 chunk-3 wo tiles in 2 waves of 8 PSUM banks
                # borrowed from the (now quiescent) existing pools — opening
                # a fresh pool here would cost a pool-transition barrier.
                # Heads 0-2 pre-accumulate while head 3's normalize (emitted
                # just above) completes; only the final h=3 matmul waits.
                qsl2 = slice(3 * 512, 4 * 512)

                def tail_bank(i, w):
                    # at most bufs-per-tag allocations per wave (a 4th sc
                    # alloc would wait its own wave's eviction -> deadlock);
                    # attnp banks last: their previous occupant (attn_ps of
                    # chunk2 h2/h3) is freed by the very norm mul this tail
                    # is overlapping, so give it the most lead time.
                    if i < 4:
                        return scp.tile([128, 512], F32, tag="sc", name=f"tail{w}_{i}")
                    if i < 6:
                        return wop.tile([128, 512], F32, tag="wo", name=f"tail{w}_{i}")
                    return attnp.tile([128, 512], F32, tag="attn", name=f"tail{w}_{i}")

                for w, wave in enumerate((range(0, 8), range(8, 16))):
                    tiles = []
                    for i, et in enumerate(wave):
                        tps_ = tail_bank(i, w)
                        for h in range(QH - 1):
                            nc.tensor.matmul(tps_[:], wos(h, et), attnT[h][:, qsl2],
                                             start=(h == 0), stop=False)
                        tiles.append((et, tps_))
                    for et, tps_ in tiles:
                        nc.tensor.matmul(tps_[:], wos(QH - 1, et),
                                         attnT[QH - 1][:, qsl2],
                                         start=False, stop=True)
                        wo_evict(3, et, tps_, qcycle=True)
    return nc


def get_nc():
    if "nc" not in _BUILT:
        nc = bass.Bass("TRN2", debug=False, enable_asserts=False,
                       num_devices=N_CORES)
        _BUILT["nc"] = _build(nc)
    return _BUILT["nc"]


def _tile_rows(w, cols):
    """[2048, cols] -> [128, 16*cols]: out[p, d*cols + j] = w[d*128+p, j]."""
    return np.ascontiguousarray(
        w.reshape(DT, 128, cols).transpose(1, 0, 2).reshape(128, DT * cols))


def prepare_in_maps(x, pos_cos, pos_sin, wq, wk, wv, wo):
    bf = ml_dtypes.bfloat16
    x = np.asarray(x, np.float32)
    pos_cos = np.asarray(pos_cos, np.float32)
    pos_sin = np.asarray(pos_sin, np.float32)
    wq = np.asarray(wq, np.float32)
    wk = np.asarray(wk, np.float32)
    wv = np.asarray(wv, np.float32)
    wo = np.asarray(wo, np.float32)

    pair = np.repeat(np.arange(HALF), 2)          # d -> d//2
    C = pos_cos.T[pair]                           # [128, S]
    Sm = pos_sin.T[pair].copy()                   # [128, S]
    Sm[0::2] *= -1.0                              # even d: -sin, odd d: +sin
    pswap = np.zeros((128, 128), np.float32)
    pswap[np.arange(128), np.arange(128) ^ 1] = 1.0
    identm = np.eye(128, dtype=np.float32)
    # maskneg[k, q] = 0 where q >= k (keep), -1e30 where q < k (mask)
    maskneg = np.where(np.triu(np.ones((128, 128), np.float32)) > 0, 0.0, -1e30)
    ones = np.ones((128, 1), np.float32)
    onerow = np.zeros((128, 128), np.float32)
    onerow[0, :] = 1.0                            # row 0 = the [1,128] ones row
    tbl = np.concatenate([pswap, identm, maskneg, ones, onerow], axis=1)  # [128, 513]

    common = {
        "cosb": C.astype(bf), "sinb": Sm.astype(np.float32),
        "tbl": tbl.astype(bf),
    }
    in_maps = []
    for c in range(N_CORES):
        b, g = divmod(c, 4)
        xt = np.ascontiguousarray(x[b].T)         # [DIM, S]
        # xtp[p, sc*8192 + d*512 + s] = xt[d*128+p, sc*512+s]
        xtp = np.ascontiguousarray(
            xt.reshape(DT, 128, NSC, 512).transpose(1, 2, 0, 3).reshape(128, -1))
        wo_g = wo[QW * g:QW * (g + 1), :]         # [512, DIM]
        wop = np.ascontiguousarray(
            wo_g.reshape(QH, 128, DIM).transpose(1, 0, 2).reshape(128, QH * DIM))
        in_maps.append(dict(
            xtp=xtp.astype(bf),
            wqp=_tile_rows(wq[:, QW * g:QW * (g + 1)], QW).astype(bf),
            wkp=_tile_rows(wk[:, KW * g:KW * (g + 1)], KW).astype(bf),
            wvp=_tile_rows(wv[:, KW * g:KW * (g + 1)], KW).astype(bf),
            wop=wop.astype(bf),
            **common,
        ))
    return in_maps


def gather(results):
    out = np.zeros((B, S, DIM), np.float32)
    for c in range(N_CORES):
        b = c // 4
        out[b] += results[c]["outT"].T.astype(np.float32)
    return out


def run(inputs, trace=False, tmpdir=None):
    nc = get_nc()
    in_maps = prepare_in_maps(**inputs)
    res = run_bass_kernel_spmd(nc, in_maps, list(range(N_CORES)),
                               trace=trace, tmpdir=tmpdir)
    return gather(res.results), res


def kernel(x, pos_cos, pos_sin, wq, wk, wv, wo):
    out, _ = run(dict(x=x, pos_cos=pos_cos, pos_sin=pos_sin,
                      wq=wq, wk=wk, wv=wv, wo=wo))
    return out


# revision 79
# speedup vs baseline: 1.0359x; 1.0359x over previous
"""GQA causal attention (RoPE) for TRN2, 8-core data+tensor parallel.

Sharding: core c in [0,8) handles batch b = c//4 and kv-head group g = c%4
(kv heads {2g, 2g+1}, q heads {4g..4g+3}).  wq/wk/wv column-sharded,
wo row-sharded by head group; host sums the 4 partial wo outputs per batch.

Device layouts (feature-major, "T" = transposed vs reference):
  xt   [DIM, S]      activations, d on partitions
  QT   [128, S]      per q head (head_dim on partitions)
  KT   [128, S]      per kv head
  V    [128k, 256]   natural (position on partitions), 16 k-tiles
  scoresT[k, q]      so softmax denominator is a partition-dim sum (ones matmul)
  attnT [128d, S]    per head -> wo matmul -> outT [DIM, S] (host transposes)

RoPE on [d, s] tiles: out = qt * C + swap_pairs(qt) * S~, with the pair swap
done by a permutation matmul on the PE and C/S~ tables prebuilt on host.

Optimizations vs the 325us baseline (now ~309us):
  - All inputs arrive as FEW BIG DMAs from host-pre-tiled DRAM layouts
    (>=4KB per-partition rows).  Per-queue DMA throughput is dispatch-
    limited (~565-667ns per dma_start) and packet-limited, so the old
    ~100-dispatch startup starved the PE for ~16us.
  - Startup: chunk 0's K/Q/V projections are interleaved per d-group so
    the PE's fresh-byte consumption (~200B/ns) stays under the DMA
    supply; its weight/xt pieces stream on BOTH HWDGE queues in exact
    consumption order (one queue sustains only ~250-280B/ns).  Later
    chunks' xt, wo, cos/sin dispatch from Act-queue program points
    anchored behind eviction copies, so they cannot steal early HBM
    bandwidth.
  - Phase-B softmax normalize: den (PE, in-stream) -> ACT Ln/Exp recip
    -> gpsimd DRAM-bounce broadcast -> DVE mul, i.e. no PE instruction
    in the chain (the old PE broadcast matmul stalled the PE ~1.2us at
    every head boundary waiting on the ACT queue).  Phase-A norms and
    the last (latency-exposed) norm keep the PE-broadcast form: the PE
    has slack there, and the gpsimd SWDGE round trip is ~6-8us.
  - wo evictions alternate ACT/DVE (a single engine queue serialized
    wop bank recycling); mid-phase outT DMAs ride the idle SP queue,
    the tail burst cycles sync/scalar/gpsimd.
  - Tail: the last chunk's 16 wo tiles run in 2 waves of 8 PSUM banks
    borrowed from the quiescent attn pools, pre-accumulating heads 0-2
    while head 3's normalize completes.

Scheduling (PE is in-order; emission order = execution order):
  - The projection phase is PE-bound while ACT/DVE idle, and the attention
    phase is bound by ACT (exp) / DVE (dac, evictions).  So attention for
    chunks 0 and 1 (and all RoPE) is broken into micro-tasks that are pumped
    between projection PSUM groups of chunks 2/3.
  - Attention chunks 2/3 run after, with scores emitted DEPTH ahead of
    their attnV matmuls, and wo tiles of ready chunks (0,1 then 3) as PE
    filler between heads to cover the exp chains.
  - Causal mask is applied by accumulating a -1e30 strict-upper block into
    the scores PSUM on the PE itself (no extra engine in the chain).
"""

import json
from collections import deque

import numpy as np
import ml_dtypes

import concourse.bass as bass
import concourse.mybir as mybir
import concourse.tile as tile
import concourse.bass2jax as bass2jax
import concourse.bass_utils as bass_utils
from concourse.bass_utils import run_bass_kernel_spmd


def _split_waits(bir_json: bytes) -> bytes:
    """This walrus build accepts at most ONE sync-wait per instruction (any
    opcode). Tile emits up to ~11. Hoist excess waits onto single-wait Drain
    fillers inserted just before the instruction on the same engine —
    same-engine program order makes this semantically identical."""
    j = json.loads(bir_json)
    changed = False
    for fn in j["functions"]:
        for b in fn["blocks"]:
            out = []
            for ins in b["instructions"]:
                si = ins.get("sync_info")
                ow = si.get("on_wait") if si else None
                if ow and len(ow) > 1:
                    changed = True
                    for k, w in enumerate(ow[:-1]):
                        out.append({
                            "debug": ins.get("debug", 0),
                            "engine": ins["engine"],
                            "ins": [], "outs": [],
                            "name": f"{ins['name']}-w{k}",
                            "opcode": "Drain",
                            "is_reset_sema": False,
                            "sync_info": {"on_update": [], "on_wait": [w]},
                        })
                    si["on_wait"] = [ow[-1]]
                out.append(ins)
            b["instructions"] = out
    return json.dumps(j).encode() if changed else bir_json


_ORIG_COMPILE = bass_utils.compile_bir_kernel


def _patched_compile(bir_json, tmpdir, neff_name="file.neff"):
    return _ORIG_COMPILE(_split_waits(bir_json), tmpdir, neff_name=neff_name)


if getattr(bass2jax.compile_bir_kernel, "__name__", "") != "_patched_compile":
    bass2jax.compile_bir_kernel = _patched_compile
    bass_utils.compile_bir_kernel = _patched_compile

BF16 = mybir.dt.bfloat16
F32 = mybir.dt.float32
Exp = mybir.ActivationFunctionType.Exp
Ln = mybir.ActivationFunctionType.Ln

B, S, DIM = 2, 2048, 2048
N_HEADS, N_KV_HEADS = 16, 8
HEAD_DIM, HALF = 128, 64
N_CORES = 8
QH, KVH = 4, 2            # q / kv heads per core
QW, KW = QH * HEAD_DIM, KVH * HEAD_DIM   # 512, 256
SCALE = 1.0 / float(np.sqrt(HEAD_DIM))

DT = DIM // 128           # 16 contraction tiles for projections
NSC = S // 512            # 4 s-chunks
NKT = S // 128            # 16 k tiles
NET = DIM // 128          # 16 output-feature tiles
DEPTH = 3                 # phase-B: score matmuls emitted ahead of attnV

_BUILT = {}


def _build(nc):
    # Pre-tiled DRAM layouts (see prepare_in_maps): every load is 1-2 big
    # DMAs with >=4KB contiguous per-partition rows.
    xtp = nc.dram_tensor("xtp", [128, NSC * DT * 512], BF16, kind="ExternalInput").ap()
    wqp = nc.dram_tensor("wqp", [128, DT * QW], BF16, kind="ExternalInput").ap()
    wkp = nc.dram_tensor("wkp", [128, DT * KW], BF16, kind="ExternalInput").ap()
    wvp = nc.dram_tensor("wvp", [128, DT * KW], BF16, kind="ExternalInput").ap()
    wop = nc.dram_tensor("wop", [128, QH * DIM], BF16, kind="ExternalInput").ap()
    cosb = nc.dram_tensor("cosb", [HEAD_DIM, S], BF16, kind="ExternalInput").ap()
    sinb = nc.dram_tensor("sinb", [HEAD_DIM, S], F32, kind="ExternalInput").ap()
    # pswp | ident | maskn | ones | onerow(row 0) merged into one table load
    tbl = nc.dram_tensor("tbl", [HEAD_DIM, 513], BF16, kind="ExternalInput").ap()
    outT = nc.dram_tensor("outT", [DIM, S], BF16, kind="ExternalOutput").ap()
    # DRAM bounce buffers for partition-broadcast of per-position reciprocals
    rscr = [nc.dram_tensor(f"rscr{i}", [1, 512], F32).ap() for i in range(NSC * QH)]

    with tile.TileContext(nc) as tc:
        with (
            tc.tile_pool(name="persist", bufs=1) as pp,
            tc.tile_pool(name="trans", bufs=2) as tp,
        ):
            # ---- startup DMA plan.  Act queue: wk halves, wv, wq, wo.
            # SP queue: xt chunk halves in chunk order (c2/c3 tile-recycle
            # gated, which is fine — they're needed much later).  Pool
            # (gpsimd SWDGE): tables + cos/sin, later the norm broadcasts.
            # Dependency tracking is tile-granular (a consumer waits on ALL
            # writers of its tile), so anything consumed piecewise gets one
            # SBUF tile per DMA: wk/wq split at d=8, xt chunks in d-quarters.
            # The SP HWDGE ring starts ~2us before Act's, so the two tensors
            # gating the first matmuls (wk half 0, xt0 quarter 0) lead the SP
            # queue; bulk weights ride Act.
            # Chunk 0's projections are interleaved K/Q/V per d-group (see
            # below) so its weights and xt arrive as d-quarter sets on the SP
            # queue in exactly consumption order: each 1.5MB set feeds ~4.3us
            # of PE, matching the ~300B/ns DMA supply.  Later chunks' xt and
            # wo ride the Act queue (its HWDGE ring starts ~2us later).
            wk_sb = [pp.tile([128, 4 * KW], BF16, tag=f"wk{g}", name=f"wk_sb{g}")
                     for g in range(4)]
            wq_sb = [pp.tile([128, 4 * QW], BF16, tag=f"wq{g}", name=f"wq_sb{g}")
                     for g in range(4)]
            wv_sb = [pp.tile([128, 4 * KW], BF16, tag=f"wv{g}", name=f"wv_sb{g}")
                     for g in range(4)]
            # Alternate the critical stream across BOTH HWDGE queues in
            # consumption order: one queue sustains only ~250-280B/ns while
            # the interleaved chunk-0 projections consume ~350B/ns.
            xa = [[None] * 4 for _ in range(NSC)]
            for g in range(4):
                nc.sync.dma_start(wk_sb[g][:], wkp[:, g * 1024:(g + 1) * 1024])
                t = tp.tile([128, 2048], BF16, tag=f"xtq{g}", bufs=2, name=f"xa0_{g}")
                nc.sync.dma_start(t[:], xtp[:, g * 2048:(g + 1) * 2048])
                xa[0][g] = t
                nc.scalar.dma_start(wq_sb[g][:], wqp[:, g * 2048:(g + 1) * 2048])
                nc.scalar.dma_start(wv_sb[g][:], wvp[:, g * 1024:(g + 1) * 1024])

            # xt chunks 1-3, wo, cos, sin are dispatched from the Act queue at
            # anchored program points inside phase A (the Act engine only
            # reaches those dma_starts after earlier copies execute), so their
            # transfers cannot steal HBM bandwidth from the critical chunk-0
            # set streaming on SP.  Tiles are allocated at the anchor points
            # to keep pool-rotation order correct.
            wo_sb = pp.tile([128, QH * DIM], BF16, tag="wo", name="wo_sb")

            def load_chunk(sc):
                for q in range(4):
                    t = tp.tile([128, 2048], BF16, tag=f"xtq{q}", bufs=2,
                                name=f"xa{sc}_{q}")
                    nc.scalar.dma_start(
                        t[:], xtp[:, sc * 8192 + q * 2048: sc * 8192 + (q + 1) * 2048])
                    xa[sc][q] = t

            tbl_sb = pp.tile([HEAD_DIM, 513], BF16, tag="tbl", name="tbl_sb")
            nc.gpsimd.dma_start(tbl_sb[:], tbl[:])
            cos_sb = pp.tile([HEAD_DIM, S], BF16, tag="cos", name="cos_sb")
            sin_sb = pp.tile([HEAD_DIM, S], F32, tag="sin", name="sin_sb")

            pswp_sb = tbl_sb[:, 0:128]
            ident_sb = tbl_sb[:, 128:256]
            maskn_sb = tbl_sb[:, 256:384]
            ones_sb = tbl_sb[:, 384:385]
            onerow_sb = tbl_sb[0:1, 385:513]

            def xs(sc, d):
                t = xa[sc][d // 4]
                return t[:, (d % 4) * 512:(d % 4 + 1) * 512]

            def wqs(d, h):
                return wq_sb[d // 4][:, (d % 4) * QW + h * 128:
                                     (d % 4) * QW + (h + 1) * 128]

            def wks(d, kv):
                return wk_sb[d // 4][:, (d % 4) * KW + kv * 128:
                                     (d % 4) * KW + (kv + 1) * 128]

            def wvs(d):
                return wv_sb[d // 4][:, (d % 4) * KW:(d % 4 + 1) * KW]

            def wos(h, et):
                return wo_sb[:, h * DIM + et * 128: h * DIM + (et + 1) * 128]

            # persistent intermediates
            qtu = [pp.tile([128, S], BF16, tag=f"qtu{h}", name=f"qtu{h}") for h in range(QH)]
            ktu = [pp.tile([128, S], BF16, tag=f"ktu{k}", name=f"ktu{k}") for k in range(KVH)]
            qtr = [pp.tile([128, S], BF16, tag=f"qtr{h}", name=f"qtr{h}") for h in range(QH)]
            ktr = [pp.tile([128, S], BF16, tag=f"ktr{k}", name=f"ktr{k}") for k in range(KVH)]
            v_sb = [pp.tile([128, KW], BF16, tag=f"v{st}", name=f"v{st}") for st in range(NKT)]
            attnT = [pp.tile([128, S], BF16, tag=f"attnT{h}", name=f"attnT{h}") for h in range(QH)]

            def norm_v3(qc, h, den_src, attn_ps):
                """normalize with no PE instruction in the chain: ACT
                1/x = exp(-ln(x)) -> gpsimd DRAM-bounce broadcast DMA ->
                DVE mul.  (DVE InstReciprocal on a [1,512] tile is ~2.6us
                of single-lane work and clogs the DVE queue; SBUF->SBUF
                partition-broadcast is illegal, so bounce through DRAM.)"""
                qsl = slice(qc * 512, (qc + 1) * 512)
                lnd = tp.tile([1, 512], F32, tag="lnd", bufs=2, name=f"lnd{qc}_{h}")
                nc.scalar.activation(lnd[:], den_src, Ln)
                recip = tp.tile([1, 512], F32, tag="recip", bufs=2, name=f"recip{qc}_{h}")
                nc.scalar.activation(recip[:], lnd[:], Exp, scale=-1.0)
                scr = rscr[qc * QH + h]
                nc.gpsimd.dma_start(scr[:], recip[:])
                rb = tp.tile([128, 512], F32, tag="rb", bufs=2, name=f"rb{qc}_{h}")
                bc = bass.AP(tensor=scr.tensor, offset=scr.offset,
                             ap=[[0, 128]] + list(scr.ap[1:]))
                nc.gpsimd.dma_start(rb[:], bc)
                nc.vector.tensor_mul(attnT[h][:, qsl], attn_ps[:], rb[:])

            # ========== Phase A: projections + interleaved attn(0,1)+rope ==
            with tc.tile_pool(name="pmA", bufs=1, space="PSUM") as pm:
                # psum/partition budget (16KB): qk 2x2K, v 2x2K(1K used),
                # shp 1x2K, asc 2x2K (scores for interleaved attn; den rides
                # row 0 of an asc tile), aps 1x2K
                tasks = deque()

                def pump(n):
                    for _ in range(n):
                        if not tasks:
                            return
                        t = tasks.popleft()
                        if next(t, None) is not None:
                            tasks.appendleft(t)

                def rope_one(src, dst, ssl, nm):
                    # (a half-swap via SBUF->SBUF DMAs was tried — it saves
                    # ~2us of PE but lengthens the rope chains that phase B's
                    # first scores wait on; the PE permutation matmul wins)
                    shp = pm.tile([128, 512], F32, tag="shp", bufs=1, name=f"shp{nm}")
                    nc.tensor.matmul(shp[:], pswp_sb, src[:, ssl], start=True, stop=True)
                    t1 = tp.tile([128, 512], BF16, tag="t1", bufs=3, name=f"rt1{nm}")
                    nc.vector.tensor_mul(t1[:], src[:, ssl], cos_sb[:, ssl])
                    t2 = tp.tile([128, 512], BF16, tag="t2", bufs=3, name=f"rt2{nm}")
                    nc.vector.tensor_mul(t2[:], shp[:], sin_sb[:, ssl])
                    nc.vector.tensor_add(dst[:, ssl], t1[:], t2[:])

                def rope_gen(sc):
                    ssl = slice(sc * 512, (sc + 1) * 512)
                    rope_one(qtu[0], qtr[0], ssl, f"q0_{sc}")
                    yield 1
                    for kv in range(KVH):
                        rope_one(ktu[kv], ktr[kv], ssl, f"k{kv}_{sc}")
                        yield 1
                    for h in range(1, QH):
                        rope_one(qtu[h], qtr[h], ssl, f"q{h}_{sc}")
                        yield 1

                def attn_gen(qc, h):
                    """micro-task generator: one kt step (score+exp+dac and
                    the previous step's attnV) per yield."""
                    qsl = slice(qc * 512, (qc + 1) * 512)
                    nkt = 4 * qc + 4
                    kv = h // 2
                    attn_ps = pm.tile([128, 512], F32, tag="aps", bufs=1, name=f"Aattn{qc}_{h}")
                    dac = tp.tile([128, 512], BF16, tag="dac", bufs=2, name=f"Adac{qc}_{h}")
                    pend = []

                    def attn_v(kt, off, span, pt):
                        nc.tensor.matmul(attn_ps[:, off:],
                                         v_sb[kt][:, kv * 128:(kv + 1) * 128],
                                         pt[:, :span], start=(kt == 0),
                                         stop=(kt == nkt - 1))

                    for kt in range(nkt):
                        off = max(0, 128 * kt - 512 * qc)
                        span = 512 - off
                        diag = kt >= 4 * qc
                        scps = pm.tile([128, 512], F32, tag="asc", bufs=2, name=f"Asc{qc}_{h}_{kt}")
                        nc.tensor.matmul(scps[:, :span], ktr[kv][:, kt * 128:(kt + 1) * 128],
                                         qtr[h][:, qc * 512 + off:(qc + 1) * 512],
                                         start=True, stop=not diag)
                        if diag:
                            nc.tensor.matmul(scps[:, :128], ident_sb, maskn_sb,
                                             start=False, stop=True)
                        pt = tp.tile([128, 512], BF16, tag="pt", bufs=6, name=f"Apt{qc}_{h}_{kt}")
                        nc.scalar.activation(pt[:, :span], scps[:, :span], Exp, scale=SCALE)
                        if kt == 0:
                            nc.vector.tensor_copy(dac[:], pt[:])
                        else:
                            nc.vector.tensor_add(dac[:, off:], dac[:, off:], pt[:, :span])
                        pend.append((kt, off, span, pt))
                        if len(pend) > 1:
                            attn_v(*pend.pop(0))
                        yield 1
                    while pend:
                        attn_v(*pend.pop(0))
                    # den rides in row 0 of an asc-tag psum tile
                    dent = pm.tile([128, 512], F32, tag="asc", bufs=2, name=f"Aden{qc}_{h}")
                    nc.tensor.matmul(dent[0:1, :], ones_sb, dac[:], start=True, stop=True)
                    yield 1
                    # phase A normalizes via PE outer-product broadcast: the
                    # PE has slack here (projections dominate), and the
                    # gpsimd DRAM bounce would serialize the end-of-phase
                    # drain where several norms flush back-to-back.
                    qsl2 = slice(qc * 512, (qc + 1) * 512)
                    lnd = tp.tile([1, 512], F32, tag="lnd", bufs=2, name=f"Alnd{qc}_{h}")
                    nc.scalar.activation(lnd[:], dent[0:1, :], Ln)
                    recipb = tp.tile([1, 512], BF16, tag="frecip", bufs=2,
                                     name=f"Afrecip{qc}_{h}")
                    nc.scalar.activation(recipb[:], lnd[:], Exp, scale=-1.0)
                    bc_ps = pm.tile([128, 512], F32, tag="asc", bufs=2, name=f"Abc{qc}_{h}")
                    nc.tensor.matmul(bc_ps[:], onerow_sb, recipb[:], start=True, stop=True)
                    rbs = tp.tile([128, 512], F32, tag="rb", bufs=2, name=f"Arbs{qc}_{h}")
                    nc.scalar.copy(rbs[:], bc_ps[:])
                    nc.vector.tensor_mul(attnT[h][:, qsl2], attn_ps[:], rbs[:])
                    yield 1

                def proj_q(sc, ssl, after=0):
                    for h in range(QH):
                        ps = pm.tile([128, 512], F32, tag="qk", bufs=2, name=f"qps{sc}_{h}")
                        for d in range(DT):
                            nc.tensor.matmul(ps[:], wqs(d, h), xs(sc, d),
                                             start=(d == 0), stop=(d == DT - 1))
                        nc.scalar.copy(qtu[h][:, ssl], ps[:])
                        pump(after)

                def proj_k(sc, ssl, after=0):
                    for kv in range(KVH):
                        ps = pm.tile([128, 512], F32, tag="qk", bufs=2, name=f"kps{sc}_{kv}")
                        for d in range(DT):
                            nc.tensor.matmul(ps[:], wks(d, kv), xs(sc, d),
                                             start=(d == 0), stop=(d == DT - 1))
                        nc.scalar.copy(ktu[kv][:, ssl], ps[:])
                        pump(after)

                def proj_v(sc, after=0):
                    for sv in range(4):
                        st = sc * 4 + sv
                        ps = pm.tile([128, KW], F32, tag="v", bufs=2, name=f"vps{st}")
                        for d in range(DT):
                            nc.tensor.matmul(ps[:], xs(sc, d)[:, sv * 128:(sv + 1) * 128],
                                             wvs(d), start=(d == 0), stop=(d == DT - 1))
                        nc.scalar.copy(v_sb[st][:], ps[:])
                        pump(after)

                # c0: interleaved K/Q(all 4 heads)/V(sv0,sv1) per d-group, so
                # the PE's consumption (~200B/ns of fresh bytes) stays below
                # the DMA supply — K alone would eat xt at ~300B/ns and
                # stall.  Accumulators borrow every free PSUM bank: K in qk,
                # Q0/Q1 in asc, Q2 in aps, Q3 in shp, V0/V1 in v (no rope or
                # attn tasks exist yet).  Pass 2 (V2,V3) re-reads SBUF xt.
                ssl0 = slice(0, 512)
                kps = [pm.tile([128, 512], F32, tag="qk", bufs=2, name=f"kps0_{kv}")
                       for kv in range(KVH)]
                qps = [pm.tile([128, 512], F32, tag="asc", bufs=2, name=f"qps0_{h}")
                       for h in range(2)]
                qps.append(pm.tile([128, 512], F32, tag="aps", bufs=1, name="qps0_2"))
                qps.append(pm.tile([128, 512], F32, tag="shp", bufs=1, name="qps0_3"))
                vps01 = [pm.tile([128, KW], F32, tag="v", bufs=2, name=f"vps0_{sv}")
                         for sv in range(2)]
                def c0_k(dg):
                    for kv in range(KVH):
                        for d in range(4 * dg, 4 * dg + 4):
                            nc.tensor.matmul(kps[kv][:], wks(d, kv), xs(0, d),
                                             start=(d == 0), stop=(d == DT - 1))

                def c0_qv(dg):
                    ds = range(4 * dg, 4 * dg + 4)
                    for h in range(QH):
                        for d in ds:
                            nc.tensor.matmul(qps[h][:], wqs(d, h), xs(0, d),
                                             start=(d == 0), stop=(d == DT - 1))
                    for sv in range(2):
                        for d in ds:
                            nc.tensor.matmul(vps01[sv][:], xs(0, d)[:, sv * 128:(sv + 1) * 128],
                                             wvs(d), start=(d == 0), stop=(d == DT - 1))

                # K for dg0+dg1 first (SP-supplied) — the Act ring starts
                # ~2.5us after SP's, so the first wq/wv arrive later; the
                # extra K work covers that window.
                c0_k(0)
                c0_k(1)
                c0_qv(0)
                c0_k(2)
                c0_qv(1)
                c0_k(3)
                c0_qv(2)
                c0_qv(3)
                for kv in range(KVH):
                    nc.scalar.copy(ktu[kv][:, ssl0], kps[kv][:])
                # anchor: Act has now executed its first copies, the SP
                # stream is nearly drained — release cos/sin + xt chunk 1.
                nc.scalar.dma_start(cos_sb[:], cosb[:])
                nc.scalar.dma_start(sin_sb[:], sinb[:])
                load_chunk(1)
                for h in range(QH):
                    nc.scalar.copy(qtu[h][:, ssl0], qps[h][:])
                for sv in range(2):
                    nc.scalar.copy(v_sb[sv][:], vps01[sv][:])
                # pass 2: V2,V3 from SBUF-resident xt
                vps23 = [pm.tile([128, KW], F32, tag="v", bufs=2, name=f"vps0_{sv}")
                         for sv in (2, 3)]
                for dg in range(4):
                    ds = range(4 * dg, 4 * dg + 4)
                    for i, sv in enumerate((2, 3)):
                        for d in ds:
                            nc.tensor.matmul(vps23[i][:], xs(0, d)[:, sv * 128:(sv + 1) * 128],
                                             wvs(d), start=(d == 0), stop=(d == DT - 1))
                for i, sv in enumerate((2, 3)):
                    nc.scalar.copy(v_sb[sv][:], vps23[i][:])
                # c1: queue rope(c0)
                tasks.append(rope_gen(0))
                load_chunk(2)
                wo_half = QH * DIM // 2
                nc.scalar.dma_start(wo_sb[:, 0:wo_half], wop[:, 0:wo_half])
                nc.scalar.dma_start(wo_sb[:, wo_half:], wop[:, wo_half:])
                ssl1 = slice(512, 1024)
                proj_q(1, ssl1, after=1)
                proj_k(1, ssl1, after=1)
                proj_v(1, after=1)
                # c2: queue rope(c1) then attn(0)
                tasks.append(rope_gen(1))
                load_chunk(3)
                for h in range(QH):
                    tasks.append(attn_gen(0, h))
                ssl2 = slice(1024, 1536)
                proj_q(2, ssl2, after=4)
                proj_k(2, ssl2, after=4)
                for h in range(QH):
                    tasks.append(attn_gen(1, h))
                proj_v(2, after=4)
                # c3: queue rope(c2) then attn(1); rope(c3) units are emitted
                # inline right after the eviction each one depends on, so the
                # DVE reaches them without queueing behind attn(1) norm muls
                # (phase B's first scores need qtr/ktr chunk 3)
                tasks.append(rope_gen(2))
                ssl3 = slice(1536, 2048)
                for h in range(QH):
                    ps = pm.tile([128, 512], F32, tag="qk", bufs=2, name=f"qps3_{h}")
                    for d in range(DT):
                        nc.tensor.matmul(ps[:], wqs(d, h), xs(3, d),
                                         start=(d == 0), stop=(d == DT - 1))
                    nc.scalar.copy(qtu[h][:, ssl3], ps[:])
                    rope_one(qtu[h], qtr[h], ssl3, f"q{h}_3")
                    pump(5)
                for kv in range(KVH):
                    ps = pm.tile([128, 512], F32, tag="qk", bufs=2, name=f"kps3_{kv}")
                    for d in range(DT):
                        nc.tensor.matmul(ps[:], wks(d, kv), xs(3, d),
                                         start=(d == 0), stop=(d == DT - 1))
                    nc.scalar.copy(ktu[kv][:, ssl3], ps[:])
                    rope_one(ktu[kv], ktr[kv], ssl3, f"k{kv}_3")
                    pump(5)
                proj_v(3, after=7)
                while tasks:
                    pump(1)

            # ========== Phase B: attn(3), attn(2) + wo tiles ==========
            wo_ctr = [0]

            def wo_evict(qc2, et, wo_ps, qcycle=False):
                # evictions alternate ACT/DVE so neither queue serializes the
                # wop bank recycling.  Mid-phase outT DMAs ride the otherwise-
                # idle SP queue (~1.1us/tile turnaround is plenty there); the
                # tail burst cycles sync/scalar/gpsimd so 16 back-to-back
                # writes don't back up the stage pool.
                qsl = slice(qc2 * 512, (qc2 + 1) * 512)
                stage = tp.tile([128, 512], BF16, tag="stage", bufs=8,
                                name=f"stage{qc2}_{et}")
                wo_ctr[0] += 1
                if wo_ctr[0] % 2:
                    nc.scalar.copy(stage[:], wo_ps[:])
                else:
                    nc.vector.tensor_copy(stage[:], wo_ps[:])
                eng = (nc.sync, nc.scalar, nc.gpsimd)[wo_ctr[0] % 3] if qcycle else nc.sync
                eng.dma_start(outT[et * 128:(et + 1) * 128, qsl], stage[:])

            with (
                tc.tile_pool(name="scp", bufs=4, space="PSUM") as scp,
                tc.tile_pool(name="attnp", bufs=2, space="PSUM") as attnp,
                tc.tile_pool(name="wop", bufs=2, space="PSUM") as wop,
            ):
                def wo_tiles(pairs, qcycle=False):
                    for qc2, et in pairs:
                        qsl = slice(qc2 * 512, (qc2 + 1) * 512)
                        wo_ps = wop.tile([128, 512], F32, tag="wo", name=f"wops{qc2}_{et}")
                        for h in range(QH):
                            nc.tensor.matmul(wo_ps[:], wos(h, et), attnT[h][:, qsl],
                                             start=(h == 0), stop=(h == QH - 1))
                        wo_evict(qc2, et, wo_ps, qcycle=qcycle)

                def attn_chunk(qc, fph, post_den=(), last=False):
                    # fph: per-head lists of (qc2, et) wo filler tiles;
                    # post_den: fillers emitted between the last head's den
                    # matmul and its normalize consumers.  Each head's
                    # normalize is deferred into the NEXT head's kt loop so
                    # the DVE mul never head-of-line-blocks the queue.
                    nkt = 4 * qc + 4
                    pending = [None]

                    def flush_norm():
                        if pending[0] is not None:
                            pending[0]()
                            pending[0] = None

                    for h in range(QH):
                        kv = h // 2
                        attn_ps = attnp.tile([128, 512], F32, tag="attn", name=f"attn{qc}_{h}")
                        dac = tp.tile([128, 512], BF16, tag="dac", bufs=2, name=f"dac{qc}_{h}")

                        def attn_v(kt, off, span, pt):
                            nc.tensor.matmul(attn_ps[:, off:],
                                             v_sb[kt][:, kv * 128:(kv + 1) * 128],
                                             pt[:, :span], start=(kt == 0),
                                             stop=(kt == nkt - 1))

                        # spread wo fillers through the kt loop, including two
                        # right at head start (kt 1 and 3): they cover the PE
                        # bubble while the first exps and the previous head's
                        # bank recycling catch up.
                        fillq = deque(fph[h])
                        stride = max(2, (nkt - 4) // max(1, max(1, len(fillq) - 2)))
                        pend = []
                        for kt in range(nkt):
                            off = max(0, 128 * kt - 512 * qc)
                            span = 512 - off
                            diag = kt >= 4 * qc
                            scps = scp.tile([128, 512], F32, tag="sc", name=f"sc{qc}_{h}_{kt}")
                            nc.tensor.matmul(scps[:, :span], ktr[kv][:, kt * 128:(kt + 1) * 128],
                                             qtr[h][:, qc * 512 + off:(qc + 1) * 512],
                                             start=True, stop=not diag)
                            if diag:
                                nc.tensor.matmul(scps[:, :128], ident_sb, maskn_sb,
                                                 start=False, stop=True)
                            pt = tp.tile([128, 512], BF16, tag="pt", bufs=6, name=f"pt{qc}_{h}_{kt}")
                            nc.scalar.activation(pt[:, :span], scps[:, :span], Exp, scale=SCALE)
                            if kt == 0:
                                nc.vector.tensor_copy(dac[:], pt[:])
                            else:
                                nc.vector.tensor_add(dac[:, off:], dac[:, off:], pt[:, :span])
                            if kt == 2:
                                flush_norm()
                            pend.append((kt, off, span, pt))
                            if len(pend) > DEPTH:
                                attn_v(*pend.pop(0))
                            if fillq and (kt == 1 or kt == 3 or
                                          (kt >= 4 and (kt - 4) % stride == 0)):
                                wo_tiles([fillq.popleft()])
                        while pend:
                            attn_v(*pend.pop(0))
                        flush_norm()

                        wo_tiles(list(fillq))

                        dent = scp.tile([128, 512], F32, tag="sc", name=f"den{qc}_{h}")
                        nc.tensor.matmul(dent[0:1, :], ones_sb, dac[:], start=True, stop=True)
                        if h == QH - 1:
                            wo_tiles(post_den)
                            if last:
                                # the very last norm is latency-exposed (the
                                # tail's h3 matmuls wait on it): PE-broadcast
                                # (~3us chain) instead of the gpsimd bounce
                                # (~8us: SWDGE gen + sem props per hop).
                                qsl3 = slice(qc * 512, (qc + 1) * 512)
                                lnd = tp.tile([1, 512], F32, tag="lnd", bufs=2,
                                              name=f"flnd{qc}_{h}")
                                nc.scalar.activation(lnd[:], dent[0:1, :], Ln)
                                recipb = tp.tile([1, 512], BF16, tag="frecip", bufs=2,
                                                 name=f"ffrecip{qc}_{h}")
                                nc.scalar.activation(recipb[:], lnd[:], Exp, scale=-1.0)
                                bc_ps = wop.tile([128, 512], F32, tag="wo",
                                                 name=f"fbc{qc}_{h}")
                                nc.tensor.matmul(bc_ps[:], onerow_sb, recipb[:],
                                                 start=True, stop=True)
                                rbs = tp.tile([128, 512], F32, tag="rb", bufs=2,
                                              name=f"frbs{qc}_{h}")
                                nc.scalar.copy(rbs[:], bc_ps[:])
                                nc.vector.tensor_mul(attnT[h][:, qsl3], attn_ps[:], rbs[:])
                            else:
                                norm_v3(qc, h, dent[0:1, :], attn_ps)
                        else:
                            def mk_norm(h=h, dent=dent, attn_ps=attn_ps):
                                norm_v3(qc, h, dent[0:1, :], attn_ps)
                            pending[0] = mk_norm

                # (chunk order 2-then-3 was tried to avoid the A->B rope
                # wait: correct but measured 3-10us slower — 3-then-2 wins)
                A32 = [(0, et) for et in range(NET)] + [(1, et) for et in range(NET)]
                B16 = [(3, et) for et in range(NET)]
                attn_chunk(3, [[], A32[0:9], A32[9:18], A32[18:26]])
                attn_chunk(2, [A32[26:32], B16[0:5], B16[5:10], B16[10:13]],
                           post_den=B16[13:16], last=True)

                # Tail: the 16 chunk-2 wo tiles in 2 waves of 8 PSUM banks
                # borrowed from the (now quiescent) existing pools — opening
                # a fresh pool here would cost a pool-transition barrier.
                # Heads 0-2 pre-accumulate while head 3's normalize (emitted
                # just above) completes; only the final h=3 matmul waits.
                qsl2 = slice(2 * 512, 3 * 512)

                def tail_bank(i, w):
                    # at most bufs-per-tag allocations per wave (a 4th sc
                    # alloc would wait its own wave's eviction -> deadlock);
                    # attnp banks last: their previous occupant (attn_ps of
                    # chunk2 h2/h3) is freed by the very norm mul this tail
                    # is overlapping, so give it the most lead time.
                    if i < 4:
                        return scp.tile([128, 512], F32, tag="sc", name=f"tail{w}_{i}")
                    if i < 6:
                        return wop.tile([128, 512], F32, tag="wo", name=f"tail{w}_{i}")
                    return attnp.tile([128, 512], F32, tag="attn", name=f"tail{w}_{i}")

                for w, wave in enumerate((range(0, 8), range(8, 16))):
                    tiles = []
                    for i, et in enumerate(wave):
                        tps_ = tail_bank(i, w)
                        for h in range(QH - 1):
                            nc.tensor.matmul(tps_[:], wos(h, et), attnT[h][:, qsl2],
                                             start=(h == 0), stop=False)
                        tiles.append((et, tps_))
                    for et, tps_ in tiles:
                        nc.tensor.matmul(tps_[:], wos(QH - 1, et),
                                         attnT[QH - 1][:, qsl2],
                                         start=False, stop=True)
                        wo_evict(2, et, tps_, qcycle=True)
    return nc


def get_nc():
    if "nc" not in _BUILT:
        nc = bass.Bass("TRN2", debug=False, enable_asserts=False,
                       num_devices=N_CORES)
        _BUILT["nc"] = _build(nc)
    return _BUILT["nc"]


def _tile_rows(w, cols):
    """[2048, cols] -> [128, 16*cols]: out[p, d*cols + j] = w[d*128+p, j]."""
    return np.ascontiguousarray(
        w.reshape(DT, 128, cols).transpose(1, 0, 2).reshape(128, DT * cols))


def prepare_in_maps(x, pos_cos, pos_sin, wq, wk, wv, wo):
    bf = ml_dtypes.bfloat16
    x = np.asarray(x, np.float32)
    pos_cos = np.asarray(pos_cos, np.float32)
    pos_sin = np.asarray(pos_sin, np.float32)
    wq = np.asarray(wq, np.float32)
    wk = np.asarray(wk, np.float32)
    wv = np.asarray(wv, np.float32)
    wo = np.asarray(wo, np.float32)

    pair = np.repeat(np.arange(HALF), 2)          # d -> d//2
    C = pos_cos.T[pair]                           # [128, S]
    Sm = pos_sin.T[pair].copy()                   # [128, S]
    Sm[0::2] *= -1.0                              # even d: -sin, odd d: +sin
    pswap = np.zeros((128, 128), np.float32)
    pswap[np.arange(128), np.arange(128) ^ 1] = 1.0
    identm = np.eye(128, dtype=np.float32)
    # maskneg[k, q] = 0 where q >= k (keep), -1e30 where q < k (mask)
    maskneg = np.where(np.triu(np.ones((128, 128), np.float32)) > 0, 0.0, -1e30)
    ones = np.ones((128, 1), np.float32)
    onerow = np.zeros((128, 128), np.float32)
    onerow[0, :] = 1.0                            # row 0 = the [1,128] ones row
    tbl = np.concatenate([pswap, identm, maskneg, ones, onerow], axis=1)  # [128, 513]

    common = {
        "cosb": C.astype(bf), "sinb": Sm.astype(np.float32),
        "tbl": tbl.astype(bf),
    }
    in_maps = []
    for c in range(N_CORES):
        b, g = divmod(c, 4)
        xt = np.ascontiguousarray(x[b].T)         # [DIM, S]
        # xtp[p, sc*8192 + d*512 + s] = xt[d*128+p, sc*512+s]
        xtp = np.ascontiguousarray(
            xt.reshape(DT, 128, NSC, 512).transpose(1, 2, 0, 3).reshape(128, -1))
        wo_g = wo[QW * g:QW * (g + 1), :]         # [512, DIM]
        wop = np.ascontiguousarray(
            wo_g.reshape(QH, 128, DIM).transpose(1, 0, 2).reshape(128, QH * DIM))
        in_maps.append(dict(
            xtp=xtp.astype(bf),
            wqp=_tile_rows(wq[:, QW * g:QW * (g + 1)], QW).astype(bf),
            wkp=_tile_rows(wk[:, KW * g:KW * (g + 1)], KW).astype(bf),
            wvp=_tile_rows(wv[:, KW * g:KW * (g + 1)], KW).astype(bf),
            wop=wop.astype(bf),
            **common,
        ))
    return in_maps


def gather(results):
    out = np.zeros((B, S, DIM), np.float32)
    for c in range(N_CORES):
        b = c // 4
        out[b] += results[c]["outT"].T.astype(np.float32)
    return out


def run(inputs, trace=False, tmpdir=None):
    nc = get_nc()
    in_maps = prepare_in_maps(**inputs)
    res = run_bass_kernel_spmd(nc, in_maps, list(range(N_CORES)),
                               trace=trace, tmpdir=tmpdir)
    return gather(res.results), res


def kernel(x, pos_cos, pos_sin, wq, wk, wv, wo):
    out, _ = run(dict(x=x, pos_cos=pos_cos, pos_sin=pos_sin,
                      wq=wq, wk=wk, wv=wv, wo=wo))
    return out


# revision 82
# speedup vs baseline: 1.0430x; 1.0069x over previous
"""GQA causal attention (RoPE) for TRN2, 8-core data+tensor parallel.

Sharding: core c in [0,8) handles batch b = c//4 and kv-head group g = c%4
(kv heads {2g, 2g+1}, q heads {4g..4g+3}).  wq/wk/wv column-sharded,
wo row-sharded by head group; host sums the 4 partial wo outputs per batch.

Device layouts (feature-major, "T" = transposed vs reference):
  xt   [DIM, S]      activations, d on partitions
  QT   [128, S]      per q head (head_dim on partitions)
  KT   [128, S]      per kv head
  V    [128k, 256]   natural (position on partitions), 16 k-tiles
  scoresT[k, q]      so softmax denominator is a partition-dim sum (ones matmul)
  attnT [128d, S]    per head -> wo matmul -> outT [DIM, S] (host transposes)

RoPE on [d, s] tiles: out = qt * C + swap_pairs(qt) * S~, with the pair swap
done by a permutation matmul on the PE and C/S~ tables prebuilt on host.

Optimizations vs the 325us baseline (now ~309us):
  - All inputs arrive as FEW BIG DMAs from host-pre-tiled DRAM layouts
    (>=4KB per-partition rows).  Per-queue DMA throughput is dispatch-
    limited (~565-667ns per dma_start) and packet-limited, so the old
    ~100-dispatch startup starved the PE for ~16us.
  - Startup: chunk 0's K/Q/V projections are interleaved per d-group so
    the PE's fresh-byte consumption (~200B/ns) stays under the DMA
    supply; its weight/xt pieces stream on BOTH HWDGE queues in exact
    consumption order (one queue sustains only ~250-280B/ns).  Later
    chunks' xt, wo, cos/sin dispatch from Act-queue program points
    anchored behind eviction copies, so they cannot steal early HBM
    bandwidth.
  - Phase-B softmax normalize: den (PE, in-stream) -> ACT Ln/Exp recip
    -> gpsimd DRAM-bounce broadcast -> DVE mul, i.e. no PE instruction
    in the chain (the old PE broadcast matmul stalled the PE ~1.2us at
    every head boundary waiting on the ACT queue).  Phase-A norms and
    the last (latency-exposed) norm keep the PE-broadcast form: the PE
    has slack there, and the gpsimd SWDGE round trip is ~6-8us.
  - wo evictions alternate ACT/DVE (a single engine queue serialized
    wop bank recycling); mid-phase outT DMAs ride the idle SP queue,
    the tail burst cycles sync/scalar/gpsimd.
  - Tail: the last chunk's 16 wo tiles run in 2 waves of 8 PSUM banks
    borrowed from the quiescent attn pools, pre-accumulating heads 0-2
    while head 3's normalize completes.

Scheduling (PE is in-order; emission order = execution order):
  - The projection phase is PE-bound while ACT/DVE idle, and the attention
    phase is bound by ACT (exp) / DVE (dac, evictions).  So attention for
    chunks 0 and 1 (and all RoPE) is broken into micro-tasks that are pumped
    between projection PSUM groups of chunks 2/3.
  - Attention chunks 2/3 run after, with scores emitted DEPTH ahead of
    their attnV matmuls, and wo tiles of ready chunks (0,1 then 3) as PE
    filler between heads to cover the exp chains.
  - Causal mask is applied by accumulating a -1e30 strict-upper block into
    the scores PSUM on the PE itself (no extra engine in the chain).
"""

import json
from collections import deque

import numpy as np
import ml_dtypes

import concourse.bass as bass
import concourse.mybir as mybir
import concourse.tile as tile
import concourse.bass2jax as bass2jax
import concourse.bass_utils as bass_utils
from concourse.bass_utils import run_bass_kernel_spmd


def _split_waits(bir_json: bytes) -> bytes:
    """This walrus build accepts at most ONE sync-wait per instruction (any
    opcode). Tile emits up to ~11. Hoist excess waits onto single-wait Drain
    fillers inserted just before the instruction on the same engine —
    same-engine program order makes this semantically identical."""
    j = json.loads(bir_json)
    changed = False
    for fn in j["functions"]:
        for b in fn["blocks"]:
            out = []
            for ins in b["instructions"]:
                si = ins.get("sync_info")
                ow = si.get("on_wait") if si else None
                if ow and len(ow) > 1:
                    changed = True
                    for k, w in enumerate(ow[:-1]):
                        out.append({
                            "debug": ins.get("debug", 0),
                            "engine": ins["engine"],
                            "ins": [], "outs": [],
                            "name": f"{ins['name']}-w{k}",
                            "opcode": "Drain",
                            "is_reset_sema": False,
                            "sync_info": {"on_update": [], "on_wait": [w]},
                        })
                    si["on_wait"] = [ow[-1]]
                out.append(ins)
            b["instructions"] = out
    return json.dumps(j).encode() if changed else bir_json


_ORIG_COMPILE = bass_utils.compile_bir_kernel


def _patched_compile(bir_json, tmpdir, neff_name="file.neff"):
    return _ORIG_COMPILE(_split_waits(bir_json), tmpdir, neff_name=neff_name)


if getattr(bass2jax.compile_bir_kernel, "__name__", "") != "_patched_compile":
    bass2jax.compile_bir_kernel = _patched_compile
    bass_utils.compile_bir_kernel = _patched_compile

BF16 = mybir.dt.bfloat16
F32 = mybir.dt.float32
Exp = mybir.ActivationFunctionType.Exp
Ln = mybir.ActivationFunctionType.Ln

B, S, DIM = 2, 2048, 2048
N_HEADS, N_KV_HEADS = 16, 8
HEAD_DIM, HALF = 128, 64
N_CORES = 8
QH, KVH = 4, 2            # q / kv heads per core
QW, KW = QH * HEAD_DIM, KVH * HEAD_DIM   # 512, 256
SCALE = 1.0 / float(np.sqrt(HEAD_DIM))

DT = DIM // 128           # 16 contraction tiles for projections
NSC = S // 512            # 4 s-chunks
NKT = S // 128            # 16 k tiles
NET = DIM // 128          # 16 output-feature tiles
DEPTH = 3                 # phase-B: score matmuls emitted ahead of attnV

_BUILT = {}


def _build(nc):
    # Pre-tiled DRAM layouts (see prepare_in_maps): every load is 1-2 big
    # DMAs with >=4KB contiguous per-partition rows.
    xtp = nc.dram_tensor("xtp", [128, NSC * DT * 512], BF16, kind="ExternalInput").ap()
    wqp = nc.dram_tensor("wqp", [128, DT * QW], BF16, kind="ExternalInput").ap()
    wkp = nc.dram_tensor("wkp", [128, DT * KW], BF16, kind="ExternalInput").ap()
    wvp = nc.dram_tensor("wvp", [128, DT * KW], BF16, kind="ExternalInput").ap()
    wop = nc.dram_tensor("wop", [128, QH * DIM], BF16, kind="ExternalInput").ap()
    cosb = nc.dram_tensor("cosb", [HEAD_DIM, S], BF16, kind="ExternalInput").ap()
    sinb = nc.dram_tensor("sinb", [HEAD_DIM, S], F32, kind="ExternalInput").ap()
    # pswp | ident | maskn | ones | onerow(row 0) merged into one table load
    tbl = nc.dram_tensor("tbl", [HEAD_DIM, 513], BF16, kind="ExternalInput").ap()
    outT = nc.dram_tensor("outT", [DIM, S], BF16, kind="ExternalOutput").ap()
    # DRAM bounce buffers for partition-broadcast of per-position reciprocals
    rscr = [nc.dram_tensor(f"rscr{i}", [1, 512], F32).ap() for i in range(NSC * QH)]

    with tile.TileContext(nc) as tc:
        with (
            tc.tile_pool(name="persist", bufs=1) as pp,
            tc.tile_pool(name="trans", bufs=2) as tp,
        ):
            # ---- startup DMA plan.  Act queue: wk halves, wv, wq, wo.
            # SP queue: xt chunk halves in chunk order (c2/c3 tile-recycle
            # gated, which is fine — they're needed much later).  Pool
            # (gpsimd SWDGE): tables + cos/sin, later the norm broadcasts.
            # Dependency tracking is tile-granular (a consumer waits on ALL
            # writers of its tile), so anything consumed piecewise gets one
            # SBUF tile per DMA: wk/wq split at d=8, xt chunks in d-quarters.
            # The SP HWDGE ring starts ~2us before Act's, so the two tensors
            # gating the first matmuls (wk half 0, xt0 quarter 0) lead the SP
            # queue; bulk weights ride Act.
            # Chunk 0's projections are interleaved K/Q/V per d-group (see
            # below) so its weights and xt arrive as d-quarter sets on the SP
            # queue in exactly consumption order: each 1.5MB set feeds ~4.3us
            # of PE, matching the ~300B/ns DMA supply.  Later chunks' xt and
            # wo ride the Act queue (its HWDGE ring starts ~2us later).
            wk_sb = [pp.tile([128, 4 * KW], BF16, tag=f"wk{g}", name=f"wk_sb{g}")
                     for g in range(4)]
            wq_sb = [pp.tile([128, 4 * QW], BF16, tag=f"wq{g}", name=f"wq_sb{g}")
                     for g in range(4)]
            wv_sb = [pp.tile([128, 4 * KW], BF16, tag=f"wv{g}", name=f"wv_sb{g}")
                     for g in range(4)]
            # Alternate the critical stream across BOTH HWDGE queues in
            # consumption order: one queue sustains only ~250-280B/ns while
            # the interleaved chunk-0 projections consume ~350B/ns.
            # chunk 0's first d-quarter is split into two eighth tiles: the
            # PE's very first matmuls then wait on a 0.25MB transfer instead
            # of 0.5MB (early DMA runs at only ~110B/ns while the engines
            # ramp), starting the stream ~2.5us earlier.
            xa = [[None] * 4 for _ in range(NSC)]
            xq0 = [tp.tile([128, 1024], BF16, tag=f"xq0{i}", bufs=1, name=f"xq0_{i}")
                   for i in range(2)]
            for g in range(4):
                nc.sync.dma_start(wk_sb[g][:], wkp[:, g * 1024:(g + 1) * 1024])
                if g == 0:
                    nc.sync.dma_start(xq0[0][:], xtp[:, 0:1024])
                    nc.sync.dma_start(xq0[1][:], xtp[:, 1024:2048])
                else:
                    t = tp.tile([128, 2048], BF16, tag=f"xtq{g}", bufs=2,
                                name=f"xa0_{g}")
                    nc.sync.dma_start(t[:], xtp[:, g * 2048:(g + 1) * 2048])
                    xa[0][g] = t
                nc.scalar.dma_start(wq_sb[g][:], wqp[:, g * 2048:(g + 1) * 2048])
                nc.scalar.dma_start(wv_sb[g][:], wvp[:, g * 1024:(g + 1) * 1024])

            # xt chunks 1-3, wo, cos, sin are dispatched from the Act queue at
            # anchored program points inside phase A (the Act engine only
            # reaches those dma_starts after earlier copies execute), so their
            # transfers cannot steal HBM bandwidth from the critical chunk-0
            # set streaming on SP.  Tiles are allocated at the anchor points
            # to keep pool-rotation order correct.
            wo_sb = pp.tile([128, QH * DIM], BF16, tag="wo", name="wo_sb")

            def load_chunk(sc):
                for q in range(4):
                    t = tp.tile([128, 2048], BF16, tag=f"xtq{q}", bufs=2,
                                name=f"xa{sc}_{q}")
                    nc.scalar.dma_start(
                        t[:], xtp[:, sc * 8192 + q * 2048: sc * 8192 + (q + 1) * 2048])
                    xa[sc][q] = t

            tbl_sb = pp.tile([HEAD_DIM, 513], BF16, tag="tbl", name="tbl_sb")
            nc.gpsimd.dma_start(tbl_sb[:], tbl[:])
            cos_sb = pp.tile([HEAD_DIM, S], BF16, tag="cos", name="cos_sb")
            sin_sb = pp.tile([HEAD_DIM, S], F32, tag="sin", name="sin_sb")

            pswp_sb = tbl_sb[:, 0:128]
            ident_sb = tbl_sb[:, 128:256]
            maskn_sb = tbl_sb[:, 256:384]
            ones_sb = tbl_sb[:, 384:385]
            onerow_sb = tbl_sb[0:1, 385:513]

            def xs(sc, d):
                if sc == 0 and d < 4:
                    t = xq0[d // 2]
                    return t[:, (d % 2) * 512:(d % 2 + 1) * 512]
                t = xa[sc][d // 4]
                return t[:, (d % 4) * 512:(d % 4 + 1) * 512]

            def wqs(d, h):
                return wq_sb[d // 4][:, (d % 4) * QW + h * 128:
                                     (d % 4) * QW + (h + 1) * 128]

            def wks(d, kv):
                return wk_sb[d // 4][:, (d % 4) * KW + kv * 128:
                                     (d % 4) * KW + (kv + 1) * 128]

            def wvs(d):
                return wv_sb[d // 4][:, (d % 4) * KW:(d % 4 + 1) * KW]

            def wos(h, et):
                return wo_sb[:, h * DIM + et * 128: h * DIM + (et + 1) * 128]

            # persistent intermediates
            qtu = [pp.tile([128, S], BF16, tag=f"qtu{h}", name=f"qtu{h}") for h in range(QH)]
            ktu = [pp.tile([128, S], BF16, tag=f"ktu{k}", name=f"ktu{k}") for k in range(KVH)]
            qtr = [pp.tile([128, S], BF16, tag=f"qtr{h}", name=f"qtr{h}") for h in range(QH)]
            ktr = [pp.tile([128, S], BF16, tag=f"ktr{k}", name=f"ktr{k}") for k in range(KVH)]
            v_sb = [pp.tile([128, KW], BF16, tag=f"v{st}", name=f"v{st}") for st in range(NKT)]
            attnT = [pp.tile([128, S], BF16, tag=f"attnT{h}", name=f"attnT{h}") for h in range(QH)]

            def norm_v3(qc, h, den_src, attn_ps):
                """normalize with no PE instruction in the chain: ACT
                1/x = exp(-ln(x)) -> gpsimd DRAM-bounce broadcast DMA ->
                DVE mul.  (DVE InstReciprocal on a [1,512] tile is ~2.6us
                of single-lane work and clogs the DVE queue; SBUF->SBUF
                partition-broadcast is illegal, so bounce through DRAM.)"""
                qsl = slice(qc * 512, (qc + 1) * 512)
                lnd = tp.tile([1, 512], F32, tag="lnd", bufs=2, name=f"lnd{qc}_{h}")
                nc.scalar.activation(lnd[:], den_src, Ln)
                recip = tp.tile([1, 512], F32, tag="recip", bufs=2, name=f"recip{qc}_{h}")
                nc.scalar.activation(recip[:], lnd[:], Exp, scale=-1.0)
                scr = rscr[qc * QH + h]
                nc.gpsimd.dma_start(scr[:], recip[:])
                rb = tp.tile([128, 512], F32, tag="rb", bufs=2, name=f"rb{qc}_{h}")
                bc = bass.AP(tensor=scr.tensor, offset=scr.offset,
                             ap=[[0, 128]] + list(scr.ap[1:]))
                nc.gpsimd.dma_start(rb[:], bc)
                nc.vector.tensor_mul(attnT[h][:, qsl], attn_ps[:], rb[:])

            # ========== Phase A: projections + interleaved attn(0,1)+rope ==
            with tc.tile_pool(name="pmA", bufs=1, space="PSUM") as pm:
                # psum/partition budget (16KB): qk 2x2K, v 2x2K(1K used),
                # shp 1x2K, asc 2x2K (scores for interleaved attn; den rides
                # row 0 of an asc tile), aps 1x2K
                tasks = deque()

                def pump(n):
                    for _ in range(n):
                        if not tasks:
                            return
                        t = tasks.popleft()
                        if next(t, None) is not None:
                            tasks.appendleft(t)

                def rope_one(src, dst, ssl, nm):
                    # (a half-swap via SBUF->SBUF DMAs was tried — it saves
                    # ~2us of PE but lengthens the rope chains that phase B's
                    # first scores wait on; the PE permutation matmul wins)
                    shp = pm.tile([128, 512], F32, tag="shp", bufs=1, name=f"shp{nm}")
                    nc.tensor.matmul(shp[:], pswp_sb, src[:, ssl], start=True, stop=True)
                    t1 = tp.tile([128, 512], BF16, tag="t1", bufs=3, name=f"rt1{nm}")
                    nc.vector.tensor_mul(t1[:], src[:, ssl], cos_sb[:, ssl])
                    t2 = tp.tile([128, 512], BF16, tag="t2", bufs=3, name=f"rt2{nm}")
                    nc.vector.tensor_mul(t2[:], shp[:], sin_sb[:, ssl])
                    nc.vector.tensor_add(dst[:, ssl], t1[:], t2[:])

                def rope_gen(sc):
                    ssl = slice(sc * 512, (sc + 1) * 512)
                    rope_one(qtu[0], qtr[0], ssl, f"q0_{sc}")
                    yield 1
                    for kv in range(KVH):
                        rope_one(ktu[kv], ktr[kv], ssl, f"k{kv}_{sc}")
                        yield 1
                    for h in range(1, QH):
                        rope_one(qtu[h], qtr[h], ssl, f"q{h}_{sc}")
                        yield 1

                def attn_gen(qc, h):
                    """micro-task generator: one kt step (score+exp+dac and
                    the previous step's attnV) per yield."""
                    qsl = slice(qc * 512, (qc + 1) * 512)
                    nkt = 4 * qc + 4
                    kv = h // 2
                    attn_ps = pm.tile([128, 512], F32, tag="aps", bufs=1, name=f"Aattn{qc}_{h}")
                    dac = tp.tile([128, 512], BF16, tag="dac", bufs=2, name=f"Adac{qc}_{h}")
                    pend = []

                    def attn_v(kt, off, span, pt):
                        nc.tensor.matmul(attn_ps[:, off:],
                                         v_sb[kt][:, kv * 128:(kv + 1) * 128],
                                         pt[:, :span], start=(kt == 0),
                                         stop=(kt == nkt - 1))

                    for kt in range(nkt):
                        off = max(0, 128 * kt - 512 * qc)
                        span = 512 - off
                        diag = kt >= 4 * qc
                        scps = pm.tile([128, 512], F32, tag="asc", bufs=2, name=f"Asc{qc}_{h}_{kt}")
                        nc.tensor.matmul(scps[:, :span], ktr[kv][:, kt * 128:(kt + 1) * 128],
                                         qtr[h][:, qc * 512 + off:(qc + 1) * 512],
                                         start=True, stop=not diag)
                        if diag:
                            nc.tensor.matmul(scps[:, :128], ident_sb, maskn_sb,
                                             start=False, stop=True)
                        pt = tp.tile([128, 512], BF16, tag="pt", bufs=6, name=f"Apt{qc}_{h}_{kt}")
                        nc.scalar.activation(pt[:, :span], scps[:, :span], Exp, scale=SCALE)
                        if kt == 0:
                            nc.vector.tensor_copy(dac[:], pt[:])
                        else:
                            nc.vector.tensor_add(dac[:, off:], dac[:, off:], pt[:, :span])
                        pend.append((kt, off, span, pt))
                        if len(pend) > 1:
                            attn_v(*pend.pop(0))
                        yield 1
                    while pend:
                        attn_v(*pend.pop(0))
                    # den rides in row 0 of an asc-tag psum tile
                    dent = pm.tile([128, 512], F32, tag="asc", bufs=2, name=f"Aden{qc}_{h}")
                    nc.tensor.matmul(dent[0:1, :], ones_sb, dac[:], start=True, stop=True)
                    yield 1
                    # phase A normalizes via PE outer-product broadcast: the
                    # PE has slack here (projections dominate), and the
                    # gpsimd DRAM bounce would serialize the end-of-phase
                    # drain where several norms flush back-to-back.
                    qsl2 = slice(qc * 512, (qc + 1) * 512)
                    lnd = tp.tile([1, 512], F32, tag="lnd", bufs=2, name=f"Alnd{qc}_{h}")
                    nc.scalar.activation(lnd[:], dent[0:1, :], Ln)
                    recipb = tp.tile([1, 512], BF16, tag="frecip", bufs=2,
                                     name=f"Afrecip{qc}_{h}")
                    nc.scalar.activation(recipb[:], lnd[:], Exp, scale=-1.0)
                    bc_ps = pm.tile([128, 512], F32, tag="asc", bufs=2, name=f"Abc{qc}_{h}")
                    nc.tensor.matmul(bc_ps[:], onerow_sb, recipb[:], start=True, stop=True)
                    rbs = tp.tile([128, 512], F32, tag="rb", bufs=2, name=f"Arbs{qc}_{h}")
                    nc.scalar.copy(rbs[:], bc_ps[:])
                    nc.vector.tensor_mul(attnT[h][:, qsl2], attn_ps[:], rbs[:])
                    yield 1

                def proj_q(sc, ssl, after=0):
                    for h in range(QH):
                        ps = pm.tile([128, 512], F32, tag="qk", bufs=2, name=f"qps{sc}_{h}")
                        for d in range(DT):
                            nc.tensor.matmul(ps[:], wqs(d, h), xs(sc, d),
                                             start=(d == 0), stop=(d == DT - 1))
                        nc.scalar.copy(qtu[h][:, ssl], ps[:])
                        pump(after)

                def proj_k(sc, ssl, after=0):
                    for kv in range(KVH):
                        ps = pm.tile([128, 512], F32, tag="qk", bufs=2, name=f"kps{sc}_{kv}")
                        for d in range(DT):
                            nc.tensor.matmul(ps[:], wks(d, kv), xs(sc, d),
                                             start=(d == 0), stop=(d == DT - 1))
                        nc.scalar.copy(ktu[kv][:, ssl], ps[:])
                        pump(after)

                def proj_v(sc, after=0):
                    for sv in range(4):
                        st = sc * 4 + sv
                        ps = pm.tile([128, KW], F32, tag="v", bufs=2, name=f"vps{st}")
                        for d in range(DT):
                            nc.tensor.matmul(ps[:], xs(sc, d)[:, sv * 128:(sv + 1) * 128],
                                             wvs(d), start=(d == 0), stop=(d == DT - 1))
                        nc.scalar.copy(v_sb[st][:], ps[:])
                        pump(after)

                # c0: interleaved K/Q(all 4 heads)/V(sv0,sv1) per d-group, so
                # the PE's consumption (~200B/ns of fresh bytes) stays below
                # the DMA supply — K alone would eat xt at ~300B/ns and
                # stall.  Accumulators borrow every free PSUM bank: K in qk,
                # Q0/Q1 in asc, Q2 in aps, Q3 in shp, V0/V1 in v (no rope or
                # attn tasks exist yet).  Pass 2 (V2,V3) re-reads SBUF xt.
                ssl0 = slice(0, 512)
                kps = [pm.tile([128, 512], F32, tag="qk", bufs=2, name=f"kps0_{kv}")
                       for kv in range(KVH)]
                qps = [pm.tile([128, 512], F32, tag="asc", bufs=2, name=f"qps0_{h}")
                       for h in range(2)]
                qps.append(pm.tile([128, 512], F32, tag="aps", bufs=1, name="qps0_2"))
                qps.append(pm.tile([128, 512], F32, tag="shp", bufs=1, name="qps0_3"))
                vps01 = [pm.tile([128, KW], F32, tag="v", bufs=2, name=f"vps0_{sv}")
                         for sv in range(2)]
                def c0_k(dg):
                    # iterate d-PAIRS so the first matmuls only depend on the
                    # first eighth tile of xt chunk 0
                    for dp in range(2):
                        for kv in range(KVH):
                            for d in (4 * dg + 2 * dp, 4 * dg + 2 * dp + 1):
                                nc.tensor.matmul(kps[kv][:], wks(d, kv), xs(0, d),
                                                 start=(d == 0), stop=(d == DT - 1))

                def c0_qv(dg):
                    ds = range(4 * dg, 4 * dg + 4)
                    for h in range(QH):
                        for d in ds:
                            nc.tensor.matmul(qps[h][:], wqs(d, h), xs(0, d),
                                             start=(d == 0), stop=(d == DT - 1))
                    for sv in range(2):
                        for d in ds:
                            nc.tensor.matmul(vps01[sv][:], xs(0, d)[:, sv * 128:(sv + 1) * 128],
                                             wvs(d), start=(d == 0), stop=(d == DT - 1))

                # K for dg0+dg1 first (SP-supplied) — the Act ring starts
                # ~2.5us after SP's, so the first wq/wv arrive later; the
                # extra K work covers that window.
                c0_k(0)
                c0_k(1)
                c0_qv(0)
                c0_k(2)
                c0_qv(1)
                c0_k(3)
                c0_qv(2)
                c0_qv(3)
                for kv in range(KVH):
                    nc.scalar.copy(ktu[kv][:, ssl0], kps[kv][:])
                # anchor: Act has now executed its first copies, the SP
                # stream is nearly drained — release cos/sin + xt chunk 1.
                nc.scalar.dma_start(cos_sb[:], cosb[:])
                nc.scalar.dma_start(sin_sb[:], sinb[:])
                load_chunk(1)
                for h in range(QH):
                    nc.scalar.copy(qtu[h][:, ssl0], qps[h][:])
                for sv in range(2):
                    nc.scalar.copy(v_sb[sv][:], vps01[sv][:])
                # pass 2: V2,V3 from SBUF-resident xt
                vps23 = [pm.tile([128, KW], F32, tag="v", bufs=2, name=f"vps0_{sv}")
                         for sv in (2, 3)]
                for dg in range(4):
                    ds = range(4 * dg, 4 * dg + 4)
                    for i, sv in enumerate((2, 3)):
                        for d in ds:
                            nc.tensor.matmul(vps23[i][:], xs(0, d)[:, sv * 128:(sv + 1) * 128],
                                             wvs(d), start=(d == 0), stop=(d == DT - 1))
                for i, sv in enumerate((2, 3)):
                    nc.scalar.copy(v_sb[sv][:], vps23[i][:])
                # c1: queue rope(c0)
                tasks.append(rope_gen(0))
                load_chunk(2)
                wo_half = QH * DIM // 2
                nc.scalar.dma_start(wo_sb[:, 0:wo_half], wop[:, 0:wo_half])
                nc.scalar.dma_start(wo_sb[:, wo_half:], wop[:, wo_half:])
                ssl1 = slice(512, 1024)
                proj_q(1, ssl1, after=1)
                proj_k(1, ssl1, after=1)
                proj_v(1, after=1)
                # c2: queue rope(c1) then attn(0)
                tasks.append(rope_gen(1))
                load_chunk(3)
                for h in range(QH):
                    tasks.append(attn_gen(0, h))
                ssl2 = slice(1024, 1536)
                proj_q(2, ssl2, after=4)
                proj_k(2, ssl2, after=4)
                for h in range(QH):
                    tasks.append(attn_gen(1, h))
                proj_v(2, after=4)
                # c3: queue rope(c2) then attn(1); rope(c3) units are emitted
                # inline right after the eviction each one depends on, so the
                # DVE reaches them without queueing behind attn(1) norm muls
                # (phase B's first scores need qtr/ktr chunk 3)
                tasks.append(rope_gen(2))
                ssl3 = slice(1536, 2048)
                for h in range(QH):
                    ps = pm.tile([128, 512], F32, tag="qk", bufs=2, name=f"qps3_{h}")
                    for d in range(DT):
                        nc.tensor.matmul(ps[:], wqs(d, h), xs(3, d),
                                         start=(d == 0), stop=(d == DT - 1))
                    nc.scalar.copy(qtu[h][:, ssl3], ps[:])
                    rope_one(qtu[h], qtr[h], ssl3, f"q{h}_3")
                    pump(5)
                for kv in range(KVH):
                    ps = pm.tile([128, 512], F32, tag="qk", bufs=2, name=f"kps3_{kv}")
                    for d in range(DT):
                        nc.tensor.matmul(ps[:], wks(d, kv), xs(3, d),
                                         start=(d == 0), stop=(d == DT - 1))
                    nc.scalar.copy(ktu[kv][:, ssl3], ps[:])
                    rope_one(ktu[kv], ktr[kv], ssl3, f"k{kv}_3")
                    pump(5)
                proj_v(3, after=7)
                while tasks:
                    pump(1)

            # ========== Phase B: attn(3), attn(2) + wo tiles ==========
            wo_ctr = [0]

            def wo_evict(qc2, et, wo_ps, qcycle=False):
                # evictions alternate ACT/DVE so neither queue serializes the
                # wop bank recycling.  Mid-phase outT DMAs ride the otherwise-
                # idle SP queue (~1.1us/tile turnaround is plenty there); the
                # tail burst cycles sync/scalar/gpsimd so 16 back-to-back
                # writes don't back up the stage pool.
                qsl = slice(qc2 * 512, (qc2 + 1) * 512)
                stage = tp.tile([128, 512], BF16, tag="stage", bufs=8,
                                name=f"stage{qc2}_{et}")
                wo_ctr[0] += 1
                if wo_ctr[0] % 2:
                    nc.scalar.copy(stage[:], wo_ps[:])
                else:
                    nc.vector.tensor_copy(stage[:], wo_ps[:])
                eng = (nc.sync, nc.scalar, nc.gpsimd)[wo_ctr[0] % 3] if qcycle else nc.sync
                eng.dma_start(outT[et * 128:(et + 1) * 128, qsl], stage[:])

            with (
                tc.tile_pool(name="scp", bufs=4, space="PSUM") as scp,
                tc.tile_pool(name="attnp", bufs=2, space="PSUM") as attnp,
                tc.tile_pool(name="wop", bufs=2, space="PSUM") as wop,
            ):
                def wo_tiles(pairs, qcycle=False):
                    for qc2, et in pairs:
                        qsl = slice(qc2 * 512, (qc2 + 1) * 512)
                        wo_ps = wop.tile([128, 512], F32, tag="wo", name=f"wops{qc2}_{et}")
                        for h in range(QH):
                            nc.tensor.matmul(wo_ps[:], wos(h, et), attnT[h][:, qsl],
                                             start=(h == 0), stop=(h == QH - 1))
                        wo_evict(qc2, et, wo_ps, qcycle=qcycle)

                def attn_chunk(qc, fph, post_den=(), last=False):
                    # fph: per-head lists of (qc2, et) wo filler tiles;
                    # post_den: fillers emitted between the last head's den
                    # matmul and its normalize consumers.  Each head's
                    # normalize is deferred into the NEXT head's kt loop so
                    # the DVE mul never head-of-line-blocks the queue.
                    nkt = 4 * qc + 4
                    pending = [None]

                    def flush_norm():
                        if pending[0] is not None:
                            pending[0]()
                            pending[0] = None

                    for h in range(QH):
                        kv = h // 2
                        attn_ps = attnp.tile([128, 512], F32, tag="attn", name=f"attn{qc}_{h}")
                        dac = tp.tile([128, 512], BF16, tag="dac", bufs=2, name=f"dac{qc}_{h}")

                        def attn_v(kt, off, span, pt):
                            nc.tensor.matmul(attn_ps[:, off:],
                                             v_sb[kt][:, kv * 128:(kv + 1) * 128],
                                             pt[:, :span], start=(kt == 0),
                                             stop=(kt == nkt - 1))

                        # spread wo fillers through the kt loop, including two
                        # right at head start (kt 1 and 3): they cover the PE
                        # bubble while the first exps and the previous head's
                        # bank recycling catch up.
                        fillq = deque(fph[h])
                        stride = max(2, (nkt - 4) // max(1, max(1, len(fillq) - 2)))
                        pend = []
                        for kt in range(nkt):
                            off = max(0, 128 * kt - 512 * qc)
                            span = 512 - off
                            diag = kt >= 4 * qc
                            scps = scp.tile([128, 512], F32, tag="sc", name=f"sc{qc}_{h}_{kt}")
                            nc.tensor.matmul(scps[:, :span], ktr[kv][:, kt * 128:(kt + 1) * 128],
                                             qtr[h][:, qc * 512 + off:(qc + 1) * 512],
                                             start=True, stop=not diag)
                            if diag:
                                nc.tensor.matmul(scps[:, :128], ident_sb, maskn_sb,
                                                 start=False, stop=True)
                            pt = tp.tile([128, 512], BF16, tag="pt", bufs=6, name=f"pt{qc}_{h}_{kt}")
                            nc.scalar.activation(pt[:, :span], scps[:, :span], Exp, scale=SCALE)
                            if kt == 0:
                                nc.vector.tensor_copy(dac[:], pt[:])
                            else:
                                nc.vector.tensor_add(dac[:, off:], dac[:, off:], pt[:, :span])
                            if kt == 2:
                                flush_norm()
                            pend.append((kt, off, span, pt))
                            if len(pend) > DEPTH:
                                attn_v(*pend.pop(0))
                            if fillq and (kt == 1 or kt == 3 or
                                          (kt >= 4 and (kt - 4) % stride == 0)):
                                wo_tiles([fillq.popleft()])
                        while pend:
                            attn_v(*pend.pop(0))
                        flush_norm()

                        wo_tiles(list(fillq))

                        dent = scp.tile([128, 512], F32, tag="sc", name=f"den{qc}_{h}")
                        nc.tensor.matmul(dent[0:1, :], ones_sb, dac[:], start=True, stop=True)
                        if h == QH - 1:
                            wo_tiles(post_den)
                            if last:
                                # the very last norm is latency-exposed (the
                                # tail's h3 matmuls wait on it): PE-broadcast
                                # (~3us chain) instead of the gpsimd bounce
                                # (~8us: SWDGE gen + sem props per hop).
                                qsl3 = slice(qc * 512, (qc + 1) * 512)
                                lnd = tp.tile([1, 512], F32, tag="lnd", bufs=2,
                                              name=f"flnd{qc}_{h}")
                                nc.scalar.activation(lnd[:], dent[0:1, :], Ln)
                                recipb = tp.tile([1, 512], BF16, tag="frecip", bufs=2,
                                                 name=f"ffrecip{qc}_{h}")
                                nc.scalar.activation(recipb[:], lnd[:], Exp, scale=-1.0)
                                bc_ps = wop.tile([128, 512], F32, tag="wo",
                                                 name=f"fbc{qc}_{h}")
                                nc.tensor.matmul(bc_ps[:], onerow_sb, recipb[:],
                                                 start=True, stop=True)
                                rbs = tp.tile([128, 512], F32, tag="rb", bufs=2,
                                              name=f"frbs{qc}_{h}")
                                nc.scalar.copy(rbs[:], bc_ps[:])
                                nc.vector.tensor_mul(attnT[h][:, qsl3], attn_ps[:], rbs[:])
                            else:
                                norm_v3(qc, h, dent[0:1, :], attn_ps)
                        else:
                            def mk_norm(h=h, dent=dent, attn_ps=attn_ps):
                                norm_v3(qc, h, dent[0:1, :], attn_ps)
                            pending[0] = mk_norm

                # (chunk order 2-then-3 was tried to avoid the A->B rope
                # wait: correct but measured 3-10us slower — 3-then-2 wins)
                A32 = [(0, et) for et in range(NET)] + [(1, et) for et in range(NET)]
                B16 = [(3, et) for et in range(NET)]
                attn_chunk(3, [[], A32[0:9], A32[9:18], A32[18:26]])
                attn_chunk(2, [A32[26:32], B16[0:5], B16[5:10], B16[10:13]],
                           post_den=B16[13:16], last=True)

                # Tail: the 16 chunk-2 wo tiles in 2 waves of 8 PSUM banks
                # borrowed from the (now quiescent) existing pools — opening
                # a fresh pool here would cost a pool-transition barrier.
                # Heads 0-2 pre-accumulate while head 3's normalize (emitted
                # just above) completes; only the final h=3 matmul waits.
                qsl2 = slice(2 * 512, 3 * 512)

                def tail_bank(i, w):
                    # at most bufs-per-tag allocations per wave (a 4th sc
                    # alloc would wait its own wave's eviction -> deadlock);
                    # attnp banks last: their previous occupant (attn_ps of
                    # chunk2 h2/h3) is freed by the very norm mul this tail
                    # is overlapping, so give it the most lead time.
                    if i < 4:
                        return scp.tile([128, 512], F32, tag="sc", name=f"tail{w}_{i}")
                    if i < 6:
                        return wop.tile([128, 512], F32, tag="wo", name=f"tail{w}_{i}")
                    return attnp.tile([128, 512], F32, tag="attn", name=f"tail{w}_{i}")

                for w, wave in enumerate((range(0, 8), range(8, 16))):
                    tiles = []
                    for i, et in enumerate(wave):
                        tps_ = tail_bank(i, w)
                        for h in range(QH - 1):
                            nc.tensor.matmul(tps_[:], wos(h, et), attnT[h][:, qsl2],
                                             start=(h == 0), stop=False)
                        tiles.append((et, tps_))
                    for et, tps_ in tiles:
                        nc.tensor.matmul(tps_[:], wos(QH - 1, et),
                                         attnT[QH - 1][:, qsl2],
                                         start=False, stop=True)
                        wo_evict(2, et, tps_, qcycle=True)
    return nc


def get_nc():
    if "nc" not in _BUILT:
        nc = bass.Bass("TRN2", debug=False, enable_asserts=False,
                       num_devices=N_CORES)
        _BUILT["nc"] = _build(nc)
    return _BUILT["nc"]


def _tile_rows(w, cols):
    """[2048, cols] -> [128, 16*cols]: out[p, d*cols + j] = w[d*128+p, j]."""
    return np.ascontiguousarray(
        w.reshape(DT, 128, cols).transpose(1, 0, 2).reshape(128, DT * cols))


def prepare_in_maps(x, pos_cos, pos_sin, wq, wk, wv, wo):
    bf = ml_dtypes.bfloat16
    x = np.asarray(x, np.float32)
    pos_cos = np.asarray(pos_cos, np.float32)
    pos_sin = np.asarray(pos_sin, np.float32)
    wq = np.asarray(wq, np.float32)
    wk = np.asarray(wk, np.float32)
    wv = np.asarray(wv, np.float32)
    wo = np.asarray(wo, np.float32)

    pair = np.repeat(np.arange(HALF), 2)          # d -> d//2
    C = pos_cos.T[pair]                           # [128, S]
    Sm = pos_sin.T[pair].copy()                   # [128, S]
    Sm[0::2] *= -1.0                              # even d: -sin, odd d: +sin
    pswap = np.zeros((128, 128), np.float32)
    pswap[np.arange(128), np.arange(128) ^ 1] = 1.0
    identm = np.eye(128, dtype=np.float32)
    # maskneg[k, q] = 0 where q >= k (keep), -1e30 where q < k (mask)
    maskneg = np.where(np.triu(np.ones((128, 128), np.float32)) > 0, 0.0, -1e30)
    ones = np.ones((128, 1), np.float32)
    onerow = np.zeros((128, 128), np.float32)
    onerow[0, :] = 1.0                            # row 0 = the [1,128] ones row
    tbl = np.concatenate([pswap, identm, maskneg, ones, onerow], axis=1)  # [128, 513]

    common = {
        "cosb": C.astype(bf), "sinb": Sm.astype(np.float32),
        "tbl": tbl.astype(bf),
    }
    in_maps = []
    for c in range(N_CORES):
        b, g = divmod(c, 4)
        xt = np.ascontiguousarray(x[b].T)         # [DIM, S]
        # xtp[p, sc*8192 + d*512 + s] = xt[d*128+p, sc*512+s]
        xtp = np.ascontiguousarray(
            xt.reshape(DT, 128, NSC, 512).transpose(1, 2, 0, 3).reshape(128, -1))
        wo_g = wo[QW * g:QW * (g + 1), :]         # [512, DIM]
        wop = np.ascontiguousarray(
            wo_g.reshape(QH, 128, DIM).transpose(1, 0, 2).reshape(128, QH * DIM))
        in_maps.append(dict(
            xtp=xtp.astype(bf),
            wqp=_tile_rows(wq[:, QW * g:QW * (g + 1)], QW).astype(bf),
            wkp=_tile_rows(wk[:, KW * g:KW * (g + 1)], KW).astype(bf),
            wvp=_tile_rows(wv[:, KW * g:KW * (g + 1)], KW).astype(bf),
            wop=wop.astype(bf),
            **common,
        ))
    return in_maps


def gather(results):
    out = np.zeros((B, S, DIM), np.float32)
    for c in range(N_CORES):
        b = c // 4
        out[b] += results[c]["outT"].T.astype(np.float32)
    return out


def run(inputs, trace=False, tmpdir=None):
    nc = get_nc()
    in_maps = prepare_in_maps(**inputs)
    res = run_bass_kernel_spmd(nc, in_maps, list(range(N_CORES)),
                               trace=trace, tmpdir=tmpdir)
    return gather(res.results), res


def kernel(x, pos_cos, pos_sin, wq, wk, wv, wo):
    out, _ = run(dict(x=x, pos_cos=pos_cos, pos_sin=pos_sin,
                      wq=wq, wk=wk, wv=wv, wo=wo))
    return out
